# revision 1
# baseline (speedup 1.0000x reference)
"""Multi-head attention (RoPE, causal) Trainium2 kernel, SPMD over 8 NeuronCores.

Problem: x[2,2048,1024] @ {W_q,W_k,W_v}[1024,1024] -> 16-head causal attention
with RoPE -> @ W_o[1024,1024].

Sharding (batch x heads): core c handles batch b=c//4 and head group g=c%4
(4 heads = 256 of the 1024 qkv dims). Each core computes its heads' QKV
projections, RoPE, causal attention, and a partial out-projection
(ctx_g @ W_o[256g:256g+256, :]). The host sums the 4 partials per batch
(unshard of a partial-sum sharding) and transposes back.

On-device layout is fully transposed ([feature, seq]) so no transposes are
needed anywhere: scores are computed as scoresT[k,q] = K^T.T @ Q^T, the
softmax denominator falls out of the AV matmul via a ones-column appended to
V, and the out-projection consumes ctxT directly.

The whole kernel is one software pipeline over seq blocks sb:
  qk-proj(cc0) -> qk-proj(cc1) -> v-proj -> attention(cc0) -> attention(cc1)
  -> [next sb's qk-proj(cc0) covers the softmax-denominator reciprocal
     chain] -> normalize + partial out-projection + store for qb=sb.
Scores for the two heads of a chunk run concurrently in the two 64-row
groups of the PE array into one [128,1024] PSUM tile, so a single ACTIVATE
exponentiates both heads. Softmax normalization is deferred: unnormalized
ctxT and denominators are staged to SBUF; per (head-pair, query-block) one
DMA repartitions the denominators to [8,128] (reciprocal cost scales with
free size only), and stride-0 DMAs from a DRAM bounce broadcast the
reciprocals across partitions. Throwaway matmuls warm the PE's HAM clock
gate during the input load and through the final normalization chain.
"""

import numpy as np
import ml_dtypes

B = 2
S = 2048
D = 1024
H = 16
HD = 64
N_CORES = 8
H_PER_CORE = 4
DQ = H_PER_CORE * HD  # 256 qkv dims per core
N_DC = D // 128  # 8 contraction chunks
N_SB = S // 512  # 4 seq blocks of 512
N_KB = S // 128  # 16 key blocks of 128
THETA = 10000.0

_CACHED = None


def _build_kernel():
    import concourse.bass as bass
    import concourse.mybir as mybir
    import concourse.tile as tile
    from concourse import bacc

    f32 = mybir.dt.float32
    bf16 = mybir.dt.bfloat16

    nc = bacc.Bacc(None, target_bir_lowering=False, num_devices=N_CORES)

    xT = nc.dram_tensor("xT", [D, S], bf16, kind="ExternalInput")
    wq = nc.dram_tensor("wq", [D, DQ], bf16, kind="ExternalInput")
    wk = nc.dram_tensor("wk", [D, DQ], bf16, kind="ExternalInput")
    wv = nc.dram_tensor("wv", [D, DQ], bf16, kind="ExternalInput")
    wo = nc.dram_tensor("wo", [DQ, D], bf16, kind="ExternalInput")
    cosT = nc.dram_tensor("cosT", [128, S], f32, kind="ExternalInput")
    sinT = nc.dram_tensor("sinT", [128, S], f32, kind="ExternalInput")
    # masks[k, 1024*j + 512*h + q] = 1.0 if (128*j + k) <= q else 0 (h=0,1 same)
    masks = nc.dram_tensor("masks", [128, 4 * 1024], bf16, kind="ExternalInput")
    yT = nc.dram_tensor("yT", [D, S], bf16, kind="ExternalOutput")

    with tile.TileContext(nc) as tc:
        with (
            tc.tile_pool(name="persist", bufs=1) as persist,
            tc.tile_pool(name="attn", bufs=8) as attn_pool,
            tc.tile_pool(name="rope", bufs=4) as rope_pool,
            tc.tile_pool(name="small", bufs=4) as small_pool,
            tc.tile_pool(name="yout", bufs=3) as yout_pool,
            tc.tile_pool(name="dram", bufs=1, space="DRAM") as dram_pool,
            tc.tile_pool(name="psA", bufs=2, space="PSUM") as psA,  # scores 2-bank
            tc.tile_pool(name="psB", bufs=2, space="PSUM") as psB,  # ctx accum
            tc.tile_pool(name="psC", bufs=2, space="PSUM") as psC,  # proj/y
        ):
            # ---------------- input DMA ----------------
            # few, large DMAs: each dma_start costs ~600ns of queue issue
            wq_sb = persist.tile([128, N_DC, DQ], bf16, tag="wq")
            nc.sync.dma_start(
                out=wq_sb[:], in_=wq.rearrange("(c p) n -> p c n", p=128)
            )
            xt_sb = [
                persist.tile([128, S], bf16, tag=f"xt{dc}", name=f"xt{dc}")
                for dc in range(N_DC)
            ]
            for dc in range(N_DC):
                eng = nc.sync if dc % 2 == 0 else nc.gpsimd
                eng.dma_start(
                    out=xt_sb[dc][:], in_=xT[128 * dc : 128 * (dc + 1), :]
                )
            wk_sb = persist.tile([128, N_DC, DQ], bf16, tag="wk")
            nc.sync.dma_start(
                out=wk_sb[:], in_=wk.rearrange("(c p) n -> p c n", p=128)
            )
            cos_sb = persist.tile([128, S], f32, tag="cos")
            sin_sb = persist.tile([128, S], f32, tag="sin")
            nc.sync.dma_start(out=cos_sb[:, 0:512], in_=cosT[:, 0:512])
            nc.sync.dma_start(out=sin_sb[:, 0:512], in_=sinT[:, 0:512])
            wv_sb = persist.tile([128, N_DC, DQ], bf16, tag="wv")
            nc.sync.dma_start(
                out=wv_sb[:], in_=wv.rearrange("(c p) n -> p c n", p=128)
            )
            nc.sync.dma_start(out=cos_sb[:, 512:S], in_=cosT[:, 512:S])
            nc.sync.dma_start(out=sin_sb[:, 512:S], in_=sinT[:, 512:S])
            mask_sb = persist.tile([128, 4 * 1024], bf16, tag="mask")
            nc.sync.dma_start(out=mask_sb[:], in_=masks[:])
            wo_sb = persist.tile([128, 2, D], bf16, tag="wo")
            nc.sync.dma_start(
                out=wo_sb[:], in_=wo.rearrange("(c p) n -> p c n", p=128)
            )

            # PE warm-up: the HAM clock gate needs ~3.4us of sustained
            # activity to lift the PE to 2.4GHz; run throwaway matmuls on the
            # first-arrived weight tile while x is still streaming in
            warm0 = psA.tile([128, DQ], f32, tag="score", name="warm0")
            for wi in range(24):
                nc.tensor.matmul(
                    warm0[:],
                    wq_sb[:, 0, 0:128],
                    wq_sb[:, wi % 4, :],
                    start=True,
                    stop=True,
                )

            # persistent intermediates
            qT_sb = persist.tile([128, 2, S], bf16, tag="qT")  # [64h..., cc, s]
            kT_sb = persist.tile([128, 2, S], bf16, tag="kT")
            v_sb = persist.tile([128, N_KB, H_PER_CORE, HD + 1], bf16, tag="v")
            nc.vector.memset(v_sb[:, :, :, HD : HD + 1], 1.0)
            ctxT_sb = persist.tile([128, 2, S], bf16, tag="ctxT")  # unnormalized
            # denominators staged on one partition (engine writes must start at
            # partition 0/32/64/96); chunk qb*4+hh holds head hh, block qb
            stage_sb = persist.tile([1, H_PER_CORE * S], f32, tag="stage")
            recip_dram = dram_pool.tile([N_SB, H_PER_CORE, 512], bf16, tag="rdram")

            # ---------------- helpers ----------------
            def rope(src_ps, dst_sb, cc, sb):
                """dst = src*cos + rotate_half(src)*sin, fp32 in, bf16 out.

                The rotate-half partition shift is done by small SBUF->SBUF
                DMAs (a [32,512] DVE op costs as much as a [128,512] one, so
                quarter-sized DVE ops waste 3/4 of the lanes; DMA engines are
                otherwise idle).
                """
                ss = slice(512 * sb, 512 * (sb + 1))
                t1 = rope_pool.tile([128, 512], bf16, tag="ropeA", name="t1")
                nc.vector.tensor_mul(t1[:], src_ps[:], cos_sb[:, ss])
                # sin table is pre-shifted on the host (sinx[p] =
                # sin_signed[partner(p)]) so this product is computed at the
                # SOURCE rows and only then moved to the partner rows by DMA
                t2p = rope_pool.tile([128, 512], bf16, tag="ropeQ", name="t2p")
                nc.vector.tensor_mul(t2p[:], src_ps[:], sin_sb[:, ss])
                rot = rope_pool.tile([128, 512], bf16, tag="ropeB", name="rot")
                for quarter in range(4):
                    o = 32 * quarter
                    src_o = o + 32 if quarter % 2 == 0 else o - 32
                    nc.gpsimd.dma_start(
                        out=rot[o : o + 32, :], in_=t2p[src_o : src_o + 32, :]
                    )
                nc.vector.tensor_add(dst_sb[:, cc, ss], t1[:], rot[:])

            def proj_qk(cc, sb):
                ss = slice(512 * sb, 512 * (sb + 1))
                q_ps = psC.tile([128, 512], f32, tag="proj", name="q_ps")
                for dc in range(N_DC):
                    nc.tensor.matmul(
                        q_ps[:],
                        wq_sb[:, dc, 128 * cc : 128 * (cc + 1)],
                        xt_sb[dc][:, ss],
                        start=(dc == 0),
                        stop=(dc == N_DC - 1),
                    )
                rope(q_ps, qT_sb, cc, sb)
                k_ps = psC.tile([128, 512], f32, tag="proj", name="k_ps")
                for dc in range(N_DC):
                    nc.tensor.matmul(
                        k_ps[:],
                        wk_sb[:, dc, 128 * cc : 128 * (cc + 1)],
                        xt_sb[dc][:, ss],
                        start=(dc == 0),
                        stop=(dc == N_DC - 1),
                    )
                rope(k_ps, kT_sb, cc, sb)

            def proj_v(sc):
                v_ps = psC.tile([128, DQ], f32, tag="proj", name="v_ps")
                for dc in range(N_DC):
                    nc.tensor.matmul(
                        v_ps[:],
                        xt_sb[dc][:, 128 * sc : 128 * (sc + 1)],
                        wv_sb[:, dc, :],
                        start=(dc == 0),
                        stop=(dc == N_DC - 1),
                    )
                nc.vector.tensor_copy(
                    v_sb[:, sc, :, 0:HD],
                    v_ps[:].rearrange("p (h d) -> p h d", h=H_PER_CORE),
                )

            def attention(cc, qb, filler=None):
                """Causal attention for head pair cc, query block qb.

                Per k-block: two score matmuls (head h in PE row-group h) into
                one [128,1024] PSUM tile, one exp over both heads, mask on
                diagonal blocks, then (one k-block delayed) the two AV
                matmuls accumulating ctx+denominator via the ones column.

                `filler` is a list of callables emitting independent PE work,
                interleaved between k-blocks to cover pipeline bubbles.
                """
                qs = slice(512 * qb, 512 * (qb + 1))
                nkb = 4 * qb + 4
                filler = list(filler or [])
                ctx_ps = [
                    psB.tile([HD + 1, 512], f32, tag="ctx", name=f"ctx{h}")
                    for h in range(2)
                ]
                pending = None  # attnT tile whose AV matmuls haven't run
                for kb in range(nkb):
                    s_ps = psA.tile([128, 1024], f32, tag="score", name="s_ps")
                    for h in range(2):
                        hp = slice(64 * h, 64 * (h + 1))
                        nc.tensor.matmul(
                            s_ps[:, 512 * h : 512 * (h + 1)],
                            kT_sb[hp, cc, 128 * kb : 128 * (kb + 1)],
                            qT_sb[hp, cc, qs],
                            start=True,
                            stop=True,
                        )
                    a_t = attn_pool.tile([128, 1024], bf16, tag="attnT", name="a_t")
                    nc.scalar.activation(
                        a_t[:],
                        s_ps[:],
                        mybir.ActivationFunctionType.Exp,
                        scale=float(1.0 / np.sqrt(HD)),
                    )
                    if kb >= 4 * qb:
                        j = kb - 4 * qb
                        nc.vector.tensor_mul(
                            a_t[:], a_t[:], mask_sb[:, 1024 * j : 1024 * (j + 1)]
                        )
                    if pending is not None:
                        pkb, p_t = pending
                        for h in range(2):
                            nc.tensor.matmul(
                                ctx_ps[h][:],
                                v_sb[:, pkb, 2 * cc + h, :],
                                p_t[:, 512 * h : 512 * (h + 1)],
                                start=(pkb == 0),
                                stop=False,
                            )
                    pending = (kb, a_t)
                pkb, p_t = pending
                for h in range(2):
                    nc.tensor.matmul(
                        ctx_ps[h][:],
                        v_sb[:, pkb, 2 * cc + h, :],
                        p_t[:, 512 * h : 512 * (h + 1)],
                        start=(pkb == 0),
                        stop=True,
                    )
                # stage denominators first (the normalization chain hangs
                # off them), then independent PE filler work to cover the
                # chain, then the bulk ctx copies
                r0 = qb * H_PER_CORE + 2 * cc
                nc.vector.tensor_copy(
                    stage_sb[0:1, 512 * r0 : 512 * (r0 + 1)],
                    ctx_ps[0][HD : HD + 1, :],
                )
                nc.scalar.copy(
                    stage_sb[0:1, 512 * (r0 + 1) : 512 * (r0 + 2)],
                    ctx_ps[1][HD : HD + 1, :],
                )
                for f in filler:
                    f()
                for h in range(2):
                    nc.vector.tensor_copy(
                        ctxT_sb[64 * h : 64 * (h + 1), cc, qs], ctx_ps[h][0:HD, :]
                    )

            def normalize(cc, qb):
                """Reciprocal + broadcast + scale for head pair cc, block qb."""
                # repartition [1, 1024] -> [8, 128] so reciprocal is cheap
                # (reciprocal cost scales with free size only)
                base = (qb * H_PER_CORE + 2 * cc) * 512
                den_q = small_pool.tile([8, 128], f32, tag="den_q", name="den_q")
                nc.sync.dma_start(
                    out=den_q[:], in_=stage_sb[0:1, base : base + 1024]
                )
                rec_q = small_pool.tile([8, 128], bf16, tag="rec_q", name="rec_q")
                with nc.allow_low_precision(
                    reason="bf16 softmax denom matches bf16 attn weights"
                ):
                    nc.vector.reciprocal(rec_q[:], den_q[:])
                if cc == 1 and qb == N_SB - 1:
                    # keep the PE's HAM clock warm through the tail
                    # normalization chain: a few scratch matmuls gated on the
                    # chain's own data so the scheduler cannot hoist them
                    warm = psA.tile([128, 512], f32, tag="score", name="warm")
                    for wi in range(8):
                        nc.tensor.matmul(
                            warm[:],
                            rec_q[:],
                            xt_sb[wi][0:8, 0:512],
                            start=True,
                            stop=True,
                        )
                nc.sync.dma_start(
                    out=recip_dram[qb, 2 * cc : 2 * cc + 2, :], in_=rec_q[:]
                )
                qs = slice(512 * qb, 512 * (qb + 1))
                bc_sb = small_pool.tile([128, 512], bf16, tag="bcast", name="bc_sb")
                for h in range(2):
                    row = recip_dram[qb, 2 * cc + h, :]
                    bcast = bass.AP(
                        tensor=row.tensor,
                        offset=row.offset,
                        ap=[[0, 64]] + list(row.ap)[-1:],
                    )
                    nc.sync.dma_start(
                        out=bc_sb[64 * h : 64 * (h + 1), :], in_=bcast
                    )
                nc.vector.tensor_mul(
                    ctxT_sb[:, cc, qs], ctxT_sb[:, cc, qs], bc_sb[:]
                )

            def out_proj(qb, ocs):
                qs = slice(512 * qb, 512 * (qb + 1))
                for oc in ocs:
                    y_ps = psC.tile([128, 512], f32, tag="proj", name="y_ps")
                    for cc in range(2):
                        nc.tensor.matmul(
                            y_ps[:],
                            wo_sb[:, cc, 128 * oc : 128 * (oc + 1)],
                            ctxT_sb[:, cc, qs],
                            start=(cc == 0),
                            stop=(cc == 1),
                        )
                    y_sb = yout_pool.tile([128, 512], bf16, tag="y", name="y_sb")
                    nc.vector.tensor_copy(y_sb[:], y_ps[:])
                    nc.sync.dma_start(
                        out=yT[128 * oc : 128 * (oc + 1), qs], in_=y_sb[:]
                    )

            # ---------------- main pipeline ----------------
            proj_qk(0, 0)
            for sb in range(N_SB):
                proj_qk(1, sb)
                for sc in range(4 * sb, 4 * sb + 4):
                    proj_v(sc)
                attention(0, sb)
                normalize(0, sb)  # chain covered by attention(1, sb) PE work
                if sb == N_SB - 1:
                    attention(
                        1,
                        sb,
                        filler=[
                            (lambda oc=oc: out_proj(2, [oc]))
                            for oc in range(4, N_DC)
                        ],
                    )
                else:
                    attention(1, sb)
                normalize(1, sb)
                if sb < N_SB - 1:
                    # emit the next block's first projection before this
                    # block's out-projection so the reciprocal chain is
                    # covered by PE work and the PE never idles
                    proj_qk(0, sb + 1)
                if sb < 2:
                    out_proj(sb, range(N_DC))
                elif sb == 2:
                    # hold back half of qb=2's out-projection; it is emitted
                    # as filler inside attention(1, 3) to cover the final
                    # normalization chain
                    out_proj(2, range(0, 4))
                else:
                    out_proj(3, range(N_DC))

    nc.compile()
    return nc


def _rope_tables():
    inv_freq = (
        1.0 / (THETA ** (np.arange(0, HD, 2, dtype=np.float32) / HD))
    ).astype(np.float32)
    pos = np.arange(S, dtype=np.float32)
    ang = pos[:, None] * inv_freq[None, :]  # [S, 32]
    cos_half = np.cos(ang).astype(np.float32).T  # [32, S]
    sin_half = np.sin(ang).astype(np.float32).T
    # per-head 64 rows: cos rows duplicated. The sin table is PRE-SHIFTED:
    # row p holds sin_signed[partner(p)] (partner = rotate-half swap), so the
    # kernel multiplies at the source rows and a plain partition-shift DMA
    # finishes rotate-half: sinx per head = (+sin | -sin).
    cos64 = np.concatenate([cos_half, cos_half], axis=0)
    sinx64 = np.concatenate([sin_half, -sin_half], axis=0)
    cosT = np.concatenate([cos64, cos64], axis=0)  # [128, S] two heads
    sinT = np.concatenate([sinx64, sinx64], axis=0)
    return np.ascontiguousarray(cosT), np.ascontiguousarray(sinT)


def _masks():
    k = np.arange(128)[:, None]
    q = np.arange(512)[None, :]
    m = np.empty((128, 4 * 1024), dtype=ml_dtypes.bfloat16)
    for j in range(4):
        blk = (128 * j + k <= q).astype(ml_dtypes.bfloat16)
        m[:, 1024 * j : 1024 * j + 512] = blk
        m[:, 1024 * j + 512 : 1024 * (j + 1)] = blk
    return m


def kernel(x, W_q, W_k, W_v, W_o):
    global _CACHED
    from concourse.bass_utils import run_bass_kernel_spmd

    if _CACHED is None:
        _CACHED = _build_kernel()
    nc = _CACHED

    bf = ml_dtypes.bfloat16
    cosT, sinT = _rope_tables()
    masks = _masks()
    x = np.asarray(x)
    W_q, W_k, W_v, W_o = (np.asarray(w) for w in (W_q, W_k, W_v, W_o))
    xT = [np.ascontiguousarray(x[b].T).astype(bf) for b in range(B)]

    in_maps = []
    for c in range(N_CORES):
        b, g = divmod(c, 4)
        cols = slice(DQ * g, DQ * (g + 1))
        in_maps.append(
            {
                "xT": xT[b],
                "wq": np.ascontiguousarray(W_q[:, cols]).astype(bf),
                "wk": np.ascontiguousarray(W_k[:, cols]).astype(bf),
                "wv": np.ascontiguousarray(W_v[:, cols]).astype(bf),
                "wo": np.ascontiguousarray(W_o[cols, :]).astype(bf),
                "cosT": cosT,
                "sinT": sinT,
                "masks": masks,
            }
        )

    res = run_bass_kernel_spmd(nc, in_maps, core_ids=list(range(N_CORES)))
    kernel.last_results = res

    y = np.empty((B, S, D), dtype=np.float32)
    for b in range(B):
        acc = res.results[4 * b]["yT"].astype(np.float32)
        for g in range(1, 4):
            acc += res.results[4 * b + g]["yT"].astype(np.float32)
        y[b] = acc.T
    return y



# revision 9
# speedup vs baseline: 1.1711x; 1.1711x over previous
"""Multi-head attention (RoPE, causal) Trainium2 kernel, SPMD over 8 NeuronCores.

Problem: x[2,2048,1024] @ {W_q,W_k,W_v}[1024,1024] -> 16-head causal attention
with RoPE -> @ W_o[1024,1024].

Sharding (batch x heads): core c handles batch b=c//4 and head group g=c%4
(4 heads = 256 of the 1024 qkv dims). Each core computes its heads' QKV
projections, RoPE, causal attention, and a partial out-projection
(ctx_g @ W_o[256g:256g+256, :]). The host sums the 4 partials per batch
(unshard of a partial-sum sharding) and transposes back.

On-device layout is fully transposed ([feature, seq]): scoresT[k,q] = K^T.T @
Q^T, the softmax denominator falls out of the AV matmul via a ones-column
appended to V, and the out-projection consumes ctxT directly.

Schedule: the exp of the attention weights runs only on the Scalar(ACT)
engine (~1 elem/lane/cycle @1.2GHz) and totals ~80us; total PE matmul work
is ~100us. The kernel therefore runs one long software-pipelined stream of
attention k-block steps (the scalar backbone) and PUMPS independent PE work
(QKV projections, out-projections, normalize chains of finished strips)
into the per-kb slack so both engines stay busy concurrently. Causal
structure is exploited at 128-column granularity: for diagonal key-blocks
only columns >= the block offset are computed in scores/exp/AV, and a single
[128,2,128] band mask handles the ragged diagonal. k-blocks run in
DESCENDING order so the final AV (full width) carries the stop flag while
the first (narrowest) carries start (start clears the whole PSUM bank, so
partial-width accumulation stays correct).
"""

import numpy as np
import ml_dtypes
from collections import deque

B = 2
S = 2048
D = 1024
H = 16
HD = 64
N_CORES = 8
H_PER_CORE = 4
DQ = H_PER_CORE * HD  # 256 qkv dims per core
N_DC = D // 128  # 8 contraction chunks
N_SB = S // 512  # 4 seq blocks of 512 (query blocks)
N_KB = S // 128  # 16 key blocks of 128
THETA = 10000.0

# ---- schedule tuning knobs (ns estimates for the pump budget model) ----
N_WARM = 7          # throwaway matmuls to lift the HAM clock gate at t=0
PUMP_SLACK = 120.0  # extra per-iteration PE budget beyond the exp/PE gap
VEC_BUDGET = 650.0  # per-iteration vector-work ceiling for pumped chunks
VEC_BUDGET_BOOST = 1300.0  # first 2 iters of a strip (slot-critical copies)
MM512 = 225.0       # warm N=512 matmul estimate
MM256 = 120.0

_CACHED = None


def _build_kernel():
    import concourse.bass as bass
    import concourse.mybir as mybir
    import concourse.tile as tile
    from concourse import bacc

    f32 = mybir.dt.float32
    bf16 = mybir.dt.bfloat16
    EXPF = mybir.ActivationFunctionType.Exp

    nc = bacc.Bacc(None, target_bir_lowering=False, num_devices=N_CORES)

    xT = nc.dram_tensor("xT", [D, S], bf16, kind="ExternalInput")
    wq = nc.dram_tensor("wq", [D, DQ], bf16, kind="ExternalInput")
    wk = nc.dram_tensor("wk", [D, DQ], bf16, kind="ExternalInput")
    wv = nc.dram_tensor("wv", [D, DQ], bf16, kind="ExternalInput")
    wo = nc.dram_tensor("wo", [DQ, D], bf16, kind="ExternalInput")
    cosT = nc.dram_tensor("cosT", [128, S], bf16, kind="ExternalInput")
    sinT = nc.dram_tensor("sinT", [128, S], bf16, kind="ExternalInput")
    # bmask[k, 128h + t] = 1.0 if k <= t else 0 (ragged diagonal band mask)
    bmask = nc.dram_tensor("bmask", [128, 2 * 128], bf16, kind="ExternalInput")
    yT = nc.dram_tensor("yT", [D, S], bf16, kind="ExternalOutput")

    with tile.TileContext(nc) as tc:
        with (
            tc.tile_pool(name="persist", bufs=1) as persist,
            tc.tile_pool(name="attn", bufs=8) as attn_pool,
            tc.tile_pool(name="rope", bufs=4) as rope_pool,
            tc.tile_pool(name="small", bufs=4) as small_pool,
            tc.tile_pool(name="yout", bufs=4) as yout_pool,
            tc.tile_pool(name="dram", bufs=1, space="DRAM") as dram_pool,
            tc.tile_pool(name="psA", bufs=2, space="PSUM") as psA,  # scores
            tc.tile_pool(name="psB", bufs=2, space="PSUM") as psB,  # ctx accum
            tc.tile_pool(name="psC", bufs=2, space="PSUM") as psC,  # proj/y
        ):
            # ---------------- PE warm-up + ACT table preload ----------------
            # The HAM clock gate needs ~3.4us of PE activity for 2.4GHz; run
            # throwaway matmuls on a memset tile from t~0.  A tiny dummy exp
            # forces the ~2.7us exp table-set load before the first real exp.
            warm_sb = persist.tile([128, 512], bf16, tag="warm")
            nc.vector.memset(warm_sb[:], 0.125)
            dummy_e = persist.tile([1, 8], bf16, tag="dummy")
            nc.scalar.activation(dummy_e[:], warm_sb[0:1, 0:8], EXPF, scale=1.0)
            for wi in range(N_WARM):
                wps = psC.tile([128, 512], f32, tag="proj", name="warm")
                nc.tensor.matmul(
                    wps[:], warm_sb[:, 0:128], warm_sb[:], start=True, stop=True
                )

            # ---------------- input DMA ----------------
            # Three parallel issue paths (two HWDGE rings + gpsimd SWDGE),
            # ordered by first-use deadline.  xt is loaded in column halves so
            # the first projections only wait on ~1.5MB.
            wq_sb = persist.tile([128, N_DC, DQ], bf16, tag="wq")
            nc.sync.dma_start(
                out=wq_sb[:], in_=wq.rearrange("(c p) n -> p c n", p=128)
            )
            wk_sb = persist.tile([128, N_DC, DQ], bf16, tag="wk")
            nc.scalar.dma_start(
                out=wk_sb[:], in_=wk.rearrange("(c p) n -> p c n", p=128)
            )
            cos_sb = persist.tile([128, S], bf16, tag="cos")
            sin_sb = persist.tile([128, S], bf16, tag="sin")
            nc.gpsimd.dma_start(out=cos_sb[:, 0:512], in_=cosT[:, 0:512])
            nc.gpsimd.dma_start(out=sin_sb[:, 0:512], in_=sinT[:, 0:512])
            bmask_sb = persist.tile([128, 2, 128], bf16, tag="bmask")
            nc.gpsimd.dma_start(
                out=bmask_sb[:], in_=bmask.rearrange("p (h t) -> p h t", h=2)
            )
            xt_sb = [
                persist.tile([128, S], bf16, tag=f"xt{dc}", name=f"xt{dc}")
                for dc in range(N_DC)
            ]
            for dc in range(4):
                nc.sync.dma_start(
                    out=xt_sb[dc][:, 0:1024],
                    in_=xT[128 * dc : 128 * (dc + 1), 0:1024],
                )
            for dc in range(4, N_DC):
                nc.scalar.dma_start(
                    out=xt_sb[dc][:, 0:1024],
                    in_=xT[128 * dc : 128 * (dc + 1), 0:1024],
                )
            wv_sb = persist.tile([128, N_DC, DQ], bf16, tag="wv")
            nc.gpsimd.dma_start(
                out=wv_sb[:], in_=wv.rearrange("(c p) n -> p c n", p=128)
            )
            nc.gpsimd.dma_start(out=cos_sb[:, 512:S], in_=cosT[:, 512:S])
            nc.gpsimd.dma_start(out=sin_sb[:, 512:S], in_=sinT[:, 512:S])
            wo_sb = persist.tile([128, 2, D], bf16, tag="wo")
            nc.scalar.dma_start(
                out=wo_sb[:], in_=wo.rearrange("(c p) n -> p c n", p=128)
            )
            for dc in range(4):
                nc.sync.dma_start(
                    out=xt_sb[dc][:, 1024:S],
                    in_=xT[128 * dc : 128 * (dc + 1), 1024:S],
                )
            for dc in range(4, N_DC):
                nc.scalar.dma_start(
                    out=xt_sb[dc][:, 1024:S],
                    in_=xT[128 * dc : 128 * (dc + 1), 1024:S],
                )

            # ---------------- persistent intermediates ----------------
            qT_sb = persist.tile([128, 2, S], bf16, tag="qT")  # [64h..., cc, s]
            kT_sb = persist.tile([128, 2, S], bf16, tag="kT")
            v_sb = persist.tile([128, N_KB, H_PER_CORE, HD + 1], bf16, tag="v")
            nc.vector.memset(v_sb[:, :, :, HD : HD + 1], 1.0)
            ctxT_sb = persist.tile([128, 2, S], bf16, tag="ctxT")  # unnormalized
            recip_dram = dram_pool.tile([N_SB, H_PER_CORE, 512], bf16, tag="rdram")

            # ---------------- pump machinery ----------------
            # pend_fast: slot-critical copies of the just-finished strip.
            # pend: FIFO of (tag, pe_ns, vec_ns, fn) chunks of independent work.
            pend_fast = deque()
            pend = deque()
            done_tags = set()

            def enq(pe, vec, fn, tag=None):
                pend.append((tag, pe, vec, fn))

            def _emit(item):
                t, _pe, _vec, fn = item
                fn()
                if t is not None:
                    done_tags.add(t)

            def drain_fast():
                while pend_fast:
                    _emit(pend_fast.popleft())

            def drain_until(tag):
                drain_fast()
                while tag not in done_tags:
                    assert pend, f"drain_until: tag {tag} not queued"
                    _emit(pend.popleft())

            def drain_all():
                drain_fast()
                while pend:
                    _emit(pend.popleft())

            def pump(pe_budget, vec_budget, force_fast=False):
                pe_s = vec_s = 0.0
                if force_fast and pend_fast:
                    # slot-critical copy of the previous strip: emit one per
                    # iteration regardless of budget so its PSUM slot frees
                    # before this strip's first AV matmuls
                    _emit(pend_fast.popleft())
                while pend_fast or pend:
                    q = pend_fast if pend_fast else pend
                    _t, pe, vec, fn = q[0]
                    if vec_s + vec > vec_budget:
                        break
                    if pe > 0.0 and pe_s >= pe_budget:
                        break
                    _emit(q.popleft())
                    pe_s += pe
                    vec_s += vec

            # ---------------- chunk generators ----------------
            DC_ORDER = [0, 4, 1, 5, 2, 6, 3, 7]  # matches DMA arrival order

            def rope_chunks(hold, dst_sb, cc, sb):
                """dst = src*cos + rotate_half(src)*sin, fp32 in, bf16 out.

                Three ~0.55us vector chunks; the partition shift is 4 small
                SBUF->SBUF DMAs on the gpsimd ring (sin is pre-shifted on the
                host so the product happens at the SOURCE rows).
                """
                ss = slice(512 * sb, 512 * (sb + 1))

                def m1():
                    hold["t1"] = rope_pool.tile([128, 512], bf16, tag="ropeA", name="t1")
                    nc.vector.tensor_mul(hold["t1"][:], hold["ps"][:], cos_sb[:, ss])

                def m2():
                    t2p = rope_pool.tile([128, 512], bf16, tag="ropeQ", name="t2p")
                    nc.vector.tensor_mul(t2p[:], hold["ps"][:], sin_sb[:, ss])
                    rot = rope_pool.tile([128, 512], bf16, tag="ropeB", name="rot")
                    for quarter in range(4):
                        o = 32 * quarter
                        so = o + 32 if quarter % 2 == 0 else o - 32
                        nc.gpsimd.dma_start(
                            out=rot[o : o + 32, :], in_=t2p[so : so + 32, :]
                        )
                    hold["rot"] = rot

                def a3():
                    nc.vector.tensor_add(
                        dst_sb[:, cc, ss], hold["t1"][:], hold["rot"][:]
                    )

                return [(0.0, 600.0, m1), (0.0, 600.0, m2), (0.0, 600.0, a3)]

            def enq_pqk(cc, sb):
                """q+k projection of head-pair cc, seq block sb, as chunks."""
                ss = slice(512 * sb, 512 * (sb + 1))
                tails = []
                for w_sb, dst in ((wq_sb, qT_sb), (wk_sb, kT_sb)):
                    hold = {}

                    def mk(pair, w_sb=w_sb, hold=hold):
                        def fn():
                            if "ps" not in hold:
                                hold["ps"] = psC.tile(
                                    [128, 512], f32, tag="proj", name="qk_ps"
                                )
                            for dc in pair:
                                nc.tensor.matmul(
                                    hold["ps"][:],
                                    w_sb[:, dc, 128 * cc : 128 * (cc + 1)],
                                    xt_sb[dc][:, ss],
                                    start=(dc == DC_ORDER[0]),
                                    stop=(dc == DC_ORDER[-1]),
                                )

                        return fn

                    for i in range(4):
                        enq(2 * MM512, 0.0, mk(tuple(DC_ORDER[2 * i : 2 * i + 2])))
                    rc = rope_chunks(hold, dst, cc, sb)
                    enq(*rc[0])
                    enq(*rc[1])
                    tails.append(rc[2])
                # the two adds last, so each rope's shift-DMA latency is
                # covered by the other tensor's projection matmuls
                enq(*tails[0])
                enq(*tails[1], tag=("pqk", cc, sb))

            def enq_pv(sb):
                """v projection for the 4 seq chunks of block sb."""
                for sc in range(4 * sb, 4 * sb + 4):
                    hold = {}

                    def mk(dcs, sc=sc, hold=hold):
                        def fn():
                            if "ps" not in hold:
                                hold["ps"] = psC.tile(
                                    [128, DQ], f32, tag="proj", name="v_ps"
                                )
                            for dc in dcs:
                                nc.tensor.matmul(
                                    hold["ps"][:],
                                    xt_sb[dc][:, 128 * sc : 128 * (sc + 1)],
                                    wv_sb[:, dc, :],
                                    start=(dc == 0),
                                    stop=(dc == N_DC - 1),
                                )

                        return fn

                    def cp(sc=sc, hold=hold):
                        nc.vector.tensor_copy(
                            v_sb[:, sc, :, 0:HD],
                            hold["ps"][:].rearrange("p (h d) -> p h d", h=H_PER_CORE),
                        )

                    enq(4 * MM256, 0.0, mk(tuple(range(4))))
                    enq(4 * MM256, 350.0, mk(tuple(range(4, 8))))
                    enq(0.0, 350.0, cp, tag=("pv", sb) if sc == 4 * sb + 3 else None)

            def op_chunk(qb, oc, tail=False):
                qs = slice(512 * qb, 512 * (qb + 1))

                def fn():
                    y_ps = psC.tile([128, 512], f32, tag="proj", name="y_ps")
                    for cci in range(2):
                        nc.tensor.matmul(
                            y_ps[:],
                            wo_sb[:, cci, 128 * oc : 128 * (oc + 1)],
                            ctxT_sb[:, cci, qs],
                            start=(cci == 0),
                            stop=(cci == 1),
                        )
                    y_sb = yout_pool.tile([128, 512], bf16, tag="y", name="y_sb")
                    if tail and oc % 2 == 1:
                        # scalar engine is idle after the last exp: use it for
                        # half the casts and the second HWDGE ring for stores
                        nc.scalar.copy(y_sb[:], y_ps[:])
                        eng = nc.scalar
                    else:
                        nc.vector.tensor_copy(y_sb[:], y_ps[:])
                        eng = nc.sync
                    eng.dma_start(out=yT[128 * oc : 128 * (oc + 1), qs], in_=y_sb[:])

                return (2 * MM512, 640.0, fn)

            def enq_op(qb):
                for oc in range(N_DC):
                    enq(*op_chunk(qb, oc))

            # ---------------- normalization ----------------
            def boundary(cc, qb, ctx, inline=False):
                """Stage denominators + evacuate ctx, then the reciprocal
                broadcast chain.  As pump chunks: the two slot-critical copy
                bundles go on the fast queue; the recip chain follows FIFO."""
                qs = slice(512 * qb, 512 * (qb + 1))
                hold = {}

                def cpy(h):
                    def fn():
                        if "stage" not in hold:
                            hold["stage"] = small_pool.tile(
                                [1, 1024], f32, tag="stage", name="stage"
                            )
                        nc.vector.tensor_copy(
                            ctxT_sb[64 * h : 64 * (h + 1), cc, qs], ctx[h][0:HD, :]
                        )
                        nc.vector.tensor_copy(
                            hold["stage"][0:1, 512 * h : 512 * (h + 1)],
                            ctx[h][HD : HD + 1, :],
                        )

                    return fn

                def n2():
                    den_q = small_pool.tile([8, 128], f32, tag="den_q", name="den_q")
                    nc.sync.dma_start(out=den_q[:], in_=hold["stage"][0:1, 0:1024])
                    rec_q = small_pool.tile([8, 128], bf16, tag="rec_q", name="rec_q")
                    with nc.allow_low_precision(
                        reason="bf16 softmax denom matches bf16 attn weights"
                    ):
                        nc.vector.reciprocal(rec_q[:], den_q[:])
                    hold["rec"] = rec_q

                def warmtail():
                    # keep the PE's HAM clock warm through the tail chain:
                    # scratch matmuls gated on the chain's own data
                    warm = psA.tile([128, 2, 512], f32, tag="score", name="warm")
                    for wi in range(8):
                        nc.tensor.matmul(
                            warm[:, 0, :],
                            hold["rec"][:],
                            xt_sb[wi][0:8, 0:512],
                            start=True,
                            stop=True,
                        )

                def n3():
                    nc.sync.dma_start(
                        out=recip_dram[qb, 2 * cc : 2 * cc + 2, :], in_=hold["rec"][:]
                    )
                    bc_sb = small_pool.tile([128, 512], bf16, tag="bcast", name="bc_sb")
                    for h in range(2):
                        row = recip_dram[qb, 2 * cc + h, :]
                        bcast = bass.AP(
                            tensor=row.tensor,
                            offset=row.offset,
                            ap=[[0, 64]] + list(row.ap)[-1:],
                        )
                        nc.sync.dma_start(
                            out=bc_sb[64 * h : 64 * (h + 1), :], in_=bcast
                        )
                    hold["bc"] = bc_sb

                def n4():
                    nc.vector.tensor_mul(
                        ctxT_sb[:, cc, qs], ctxT_sb[:, cc, qs], hold["bc"][:]
                    )

                if inline:
                    cpy(0)()
                    cpy(1)()
                    n2()
                    warmtail()
                    n3()
                    n4()
                else:
                    pend_fast.append((None, 0.0, 1200.0, cpy(0)))
                    pend_fast.append((None, 0.0, 1200.0, cpy(1)))
                    enq(0.0, 200.0, n2)
                    enq(0.0, 0.0, n3)
                    enq(0.0, 600.0, n4)

            # ---------------- attention strip ----------------
            def exp_cost(off):
                return (2 * (512 - off) + 352) / 1.2

            def attention(cc, qb):
                """Causal attention for head pair cc, query block qb.

                Per k-block (descending): two concurrent score matmuls (head h
                in PE row-group h) into one [128,2,512] PSUM tile, one exp over
                both heads' live columns, band-mask on diagonal blocks, and the
                AV matmuls two iterations delayed so the exp latency is hidden.
                Independent PE work is pumped into the per-iteration slack."""
                qs0 = 512 * qb
                nkb = 4 * qb + 4
                ctx = [
                    psB.tile([HD + 1, 512], f32, tag="ctx", name=f"ctx{h}")
                    for h in range(2)
                ]
                pend_av = deque()
                state = {"first": True}

                def flush_av(last=False):
                    pkb, poff, p_t = pend_av.popleft()
                    for h in range(2):
                        nc.tensor.matmul(
                            ctx[h][:, poff:512],
                            v_sb[:, pkb, 2 * cc + h, :],
                            p_t[:, h, poff:512],
                            start=state["first"],
                            stop=last,
                        )
                    state["first"] = False
                    return poff

                for idx, kb in enumerate(range(nkb - 1, -1, -1)):
                    j = kb - 4 * qb
                    off = 128 * j if j > 0 else 0
                    s_ps = psA.tile([128, 2, 512], f32, tag="score", name="s_ps")
                    for h in range(2):
                        hp = slice(64 * h, 64 * (h + 1))
                        nc.tensor.matmul(
                            s_ps[:, h, off:512],
                            kT_sb[hp, cc, 128 * kb : 128 * (kb + 1)],
                            qT_sb[hp, cc, qs0 + off : qs0 + 512],
                            start=True,
                            stop=True,
                        )
                    a_t = attn_pool.tile([128, 2, 512], bf16, tag="attnT", name="a_t")
                    nc.scalar.activation(
                        a_t[:, :, off:512],
                        s_ps[:, :, off:512],
                        EXPF,
                        scale=float(1.0 / np.sqrt(HD)),
                    )
                    if j >= 0:
                        nc.vector.tensor_mul(
                            a_t[:, :, off : off + 128],
                            a_t[:, :, off : off + 128],
                            bmask_sb[:],
                        )
                    pend_av.append((kb, off, a_t))
                    pe_used = (512 - off) / 2.4 + 90.0
                    if idx >= 2:
                        poff = flush_av()
                        pe_used += 2 * (512 - poff) / 2.4 + 60.0
                    budget = exp_cost(off) - pe_used + PUMP_SLACK
                    vec_b = VEC_BUDGET - (200.0 if j >= 0 else 0.0)
                    pump(budget, vec_b, force_fast=(idx < 2))
                flush_av()
                flush_av(last=True)
                return ctx

            # ---------------- main schedule ----------------
            enq_pqk(0, 0)
            drain_all()  # first projection inline (nothing to overlap yet)
            enq_pv(0)
            enq_pqk(1, 0)
            enq_pqk(0, 1)

            drain_until(("pv", 0))
            ctx = attention(0, 0)
            boundary(0, 0, ctx)
            enq_pv(1)
            enq_pqk(1, 1)

            drain_until(("pqk", 1, 0))
            ctx = attention(1, 0)
            boundary(1, 0, ctx)
            enq_pqk(0, 2)

            drain_until(("pqk", 0, 1))
            drain_until(("pv", 1))
            ctx = attention(0, 1)
            boundary(0, 1, ctx)
            enq_pv(2)
            enq_pqk(1, 2)

            drain_until(("pqk", 1, 1))
            ctx = attention(1, 1)
            boundary(1, 1, ctx)
            enq_op(0)
            enq_pqk(0, 3)

            drain_until(("pqk", 0, 2))
            drain_until(("pv", 2))
            ctx = attention(0, 2)
            boundary(0, 2, ctx)
            enq_pv(3)
            enq_pqk(1, 3)

            drain_until(("pqk", 1, 2))
            ctx = attention(1, 2)
            boundary(1, 2, ctx)
            enq_op(1)

            drain_until(("pqk", 0, 3))
            drain_until(("pv", 3))
            ctx = attention(0, 3)
            boundary(0, 3, ctx)
            enq_op(2)

            drain_until(("pqk", 1, 3))
            ctx = attention(1, 3)
            drain_all()
            boundary(1, 3, ctx, inline=True)
            for oc in range(N_DC):
                op_chunk(3, oc, tail=True)[2]()

    nc.compile()
    return nc


def _rope_tables():
    inv_freq = (
        1.0 / (THETA ** (np.arange(0, HD, 2, dtype=np.float32) / HD))
    ).astype(np.float32)
    pos = np.arange(S, dtype=np.float32)
    ang = pos[:, None] * inv_freq[None, :]  # [S, 32]
    cos_half = np.cos(ang).astype(np.float32).T  # [32, S]
    sin_half = np.sin(ang).astype(np.float32).T
    # per-head 64 rows: cos rows duplicated. The sin table is PRE-SHIFTED:
    # row p holds sin_signed[partner(p)] (partner = rotate-half swap), so the
    # kernel multiplies at the source rows and a plain partition-shift DMA
    # finishes rotate-half: sinx per head = (+sin | -sin).
    cos64 = np.concatenate([cos_half, cos_half], axis=0)
    sinx64 = np.concatenate([sin_half, -sin_half], axis=0)
    cosT = np.concatenate([cos64, cos64], axis=0)  # [128, S] two heads
    sinT = np.concatenate([sinx64, sinx64], axis=0)
    bf = ml_dtypes.bfloat16
    return (
        np.ascontiguousarray(cosT).astype(bf),
        np.ascontiguousarray(sinT).astype(bf),
    )


def _bmask():
    k = np.arange(128)[:, None]
    t = np.arange(128)[None, :]
    blk = (k <= t).astype(ml_dtypes.bfloat16)
    return np.ascontiguousarray(np.concatenate([blk, blk], axis=1))


def kernel(x, W_q, W_k, W_v, W_o):
    global _CACHED
    from concourse.bass_utils import run_bass_kernel_spmd

    if _CACHED is None:
        _CACHED = _build_kernel()
    nc = _CACHED

    bf = ml_dtypes.bfloat16
    cosT, sinT = _rope_tables()
    bmask = _bmask()
    x = np.asarray(x)
    W_q, W_k, W_v, W_o = (np.asarray(w) for w in (W_q, W_k, W_v, W_o))
    xT = [np.ascontiguousarray(x[b].T).astype(bf) for b in range(B)]

    in_maps = []
    for c in range(N_CORES):
        b, g = divmod(c, 4)
        cols = slice(DQ * g, DQ * (g + 1))
        in_maps.append(
            {
                "xT": xT[b],
                "wq": np.ascontiguousarray(W_q[:, cols]).astype(bf),
                "wk": np.ascontiguousarray(W_k[:, cols]).astype(bf),
                "wv": np.ascontiguousarray(W_v[:, cols]).astype(bf),
                "wo": np.ascontiguousarray(W_o[cols, :]).astype(bf),
                "cosT": cosT,
                "sinT": sinT,
                "bmask": bmask,
            }
        )

    res = run_bass_kernel_spmd(nc, in_maps, core_ids=list(range(N_CORES)))
    kernel.last_results = res

    y = np.empty((B, S, D), dtype=np.float32)
    for b in range(B):
        acc = res.results[4 * b]["yT"].astype(np.float32)
        for g in range(1, 4):
            acc += res.results[4 * b + g]["yT"].astype(np.float32)
        y[b] = acc.T
    return y


# revision 18
# speedup vs baseline: 1.1753x; 1.0036x over previous
"""Multi-head attention (RoPE, causal) Trainium2 kernel, SPMD over 8 NeuronCores.

Problem: x[2,2048,1024] @ {W_q,W_k,W_v}[1024,1024] -> 16-head causal attention
with RoPE -> @ W_o[1024,1024].

Sharding (batch x heads): core c handles batch b=c//4 and head group g=c%4
(4 heads = 256 of the 1024 qkv dims). Each core computes its heads' QKV
projections, RoPE, causal attention, and a partial out-projection
(ctx_g @ W_o[256g:256g+256, :]). The host sums the 4 partials per batch
(unshard of a partial-sum sharding) and transposes back.

On-device layout is fully transposed ([feature, seq]): scoresT[k,q] = K^T.T @
Q^T, the softmax denominator falls out of the AV matmul via a ones-column
appended to V, and the out-projection consumes ctxT directly.

Schedule: the exp of the attention weights runs only on the Scalar(ACT)
engine (~1 elem/lane/cycle @1.2GHz) and totals ~80us; total PE matmul work
is ~100us. The kernel therefore runs one long software-pipelined stream of
attention k-block steps (the scalar backbone) and PUMPS independent PE work
(QKV projections, out-projections, normalize chains of finished strips)
into the per-kb slack so both engines stay busy concurrently. Causal
structure is exploited at 128-column granularity: for diagonal key-blocks
only columns >= the block offset are computed in scores/exp/AV, and a single
[128,2,128] band mask handles the ragged diagonal. k-blocks run in
DESCENDING order so the final AV (full width) carries the stop flag while
the first (narrowest) carries start (start clears the whole PSUM bank, so
partial-width accumulation stays correct).
"""

import numpy as np
import ml_dtypes
from collections import deque

B = 2
S = 2048
D = 1024
H = 16
HD = 64
N_CORES = 8
H_PER_CORE = 4
DQ = H_PER_CORE * HD  # 256 qkv dims per core
N_DC = D // 128  # 8 contraction chunks
N_SB = S // 512  # 4 seq blocks of 512 (query blocks)
N_KB = S // 128  # 16 key blocks of 128
THETA = 10000.0

# ---- schedule tuning knobs (ns estimates for the pump budget model) ----
N_WARM = 7          # throwaway matmuls to lift the HAM clock gate at t=0
PUMP_SLACK = 120.0  # extra per-iteration PE budget beyond the exp/PE gap
VEC_BUDGET = 650.0  # per-iteration vector-work ceiling for pumped chunks
VEC_BUDGET_BOOST = 1300.0  # first 2 iters of a strip (slot-critical copies)
MM512 = 225.0       # warm N=512 matmul estimate
MM256 = 120.0

_CACHED = None


def _build_kernel():
    import concourse.bass as bass
    import concourse.mybir as mybir
    import concourse.tile as tile
    from concourse import bacc

    f32 = mybir.dt.float32
    bf16 = mybir.dt.bfloat16
    EXPF = mybir.ActivationFunctionType.Exp

    nc = bacc.Bacc(None, target_bir_lowering=False, num_devices=N_CORES)

    xT = nc.dram_tensor("xT", [D, S], bf16, kind="ExternalInput")
    # weights are pre-swizzled on the host to [partition, chunk*cols] so the
    # input DMAs are fully contiguous per partition (512B descriptors from a
    # rearrange on the device side measured ~3x slower)
    wq = nc.dram_tensor("wq", [128, N_DC * DQ], bf16, kind="ExternalInput")
    wk = nc.dram_tensor("wk", [128, N_DC * DQ], bf16, kind="ExternalInput")
    wv = nc.dram_tensor("wv", [128, N_DC * DQ], bf16, kind="ExternalInput")
    wo = nc.dram_tensor("wo", [128, 2 * D], bf16, kind="ExternalInput")
    cosT = nc.dram_tensor("cosT", [128, S], bf16, kind="ExternalInput")
    sinT = nc.dram_tensor("sinT", [128, S], bf16, kind="ExternalInput")
    # bmask[k, 128h + t] = 1.0 if k <= t else 0 (ragged diagonal band mask)
    bmask = nc.dram_tensor("bmask", [128, 2 * 128], bf16, kind="ExternalInput")
    yT = nc.dram_tensor("yT", [D, S], bf16, kind="ExternalOutput")

    with tile.TileContext(nc) as tc:
        with (
            tc.tile_pool(name="persist", bufs=1) as persist,
            tc.tile_pool(name="attn", bufs=8) as attn_pool,
            tc.tile_pool(name="rope", bufs=4) as rope_pool,
            tc.tile_pool(name="small", bufs=4) as small_pool,
            tc.tile_pool(name="yout", bufs=4) as yout_pool,
            tc.tile_pool(name="dram", bufs=1, space="DRAM") as dram_pool,
            tc.tile_pool(name="psA", bufs=2, space="PSUM") as psA,  # scores
            tc.tile_pool(name="psB", bufs=2, space="PSUM") as psB,  # ctx accum
            tc.tile_pool(name="psC", bufs=2, space="PSUM") as psC,  # proj/y
        ):
            # ---------------- PE warm-up + ACT table preload ----------------
            # The HAM clock gate needs ~3.4us of PE activity for 2.4GHz; run
            # throwaway matmuls on a memset tile from t~0.  A tiny dummy exp
            # forces the ~2.7us exp table-set load before the first real exp.
            warm_sb = persist.tile([128, 512], bf16, tag="warm")
            nc.gpsimd.memset(warm_sb[:], 0.125)
            dummy_e = persist.tile([1, 8], bf16, tag="dummy")
            nc.scalar.activation(dummy_e[:], warm_sb[0:1, 0:8], EXPF, scale=1.0)
            for wi in range(N_WARM):
                wps = psC.tile([128, 512], f32, tag="proj", name="warm")
                nc.tensor.matmul(
                    wps[:], warm_sb[:, 0:128], warm_sb[:], start=True, stop=True
                )

            # ---------------- input DMA ----------------
            # Three parallel issue paths (two HWDGE rings + gpsimd SWDGE),
            # ordered by first-use deadline.  xt is loaded in column halves
            # and the weights in chunk halves so the first projection matmuls
            # only wait on a few hundred KB per ring.
            wq_sb = persist.tile([128, N_DC, DQ], bf16, tag="wq")
            wk_sb = persist.tile([128, N_DC, DQ], bf16, tag="wk")
            xt_sb = [
                persist.tile([128, S], bf16, tag=f"xt{dc}", name=f"xt{dc}")
                for dc in range(N_DC)
            ]
            nc.sync.dma_start(out=wq_sb[:, 0:4, :], in_=wq[:, 0 : 4 * DQ])
            nc.scalar.dma_start(out=wk_sb[:, 0:4, :], in_=wk[:, 0 : 4 * DQ])
            for dc in range(2):
                nc.sync.dma_start(
                    out=xt_sb[dc][:, 0:1024],
                    in_=xT[128 * dc : 128 * (dc + 1), 0:1024],
                )
            for dc in range(4, 6):
                nc.scalar.dma_start(
                    out=xt_sb[dc][:, 0:1024],
                    in_=xT[128 * dc : 128 * (dc + 1), 0:1024],
                )
            nc.sync.dma_start(out=wq_sb[:, 4:8, :], in_=wq[:, 4 * DQ : 8 * DQ])
            nc.scalar.dma_start(out=wk_sb[:, 4:8, :], in_=wk[:, 4 * DQ : 8 * DQ])
            for dc in range(2, 4):
                nc.sync.dma_start(
                    out=xt_sb[dc][:, 0:1024],
                    in_=xT[128 * dc : 128 * (dc + 1), 0:1024],
                )
            for dc in range(6, 8):
                nc.scalar.dma_start(
                    out=xt_sb[dc][:, 0:1024],
                    in_=xT[128 * dc : 128 * (dc + 1), 0:1024],
                )
            cos_sb = persist.tile([128, S], bf16, tag="cos")
            sin_sb = persist.tile([128, S], bf16, tag="sin")
            nc.gpsimd.dma_start(out=cos_sb[:, 0:512], in_=cosT[:, 0:512])
            nc.gpsimd.dma_start(out=sin_sb[:, 0:512], in_=sinT[:, 0:512])
            bmask_sb = persist.tile([128, 2, 128], bf16, tag="bmask")
            nc.gpsimd.dma_start(
                out=bmask_sb[:], in_=bmask.rearrange("p (h t) -> p h t", h=2)
            )
            wv_sb = persist.tile([128, N_DC, DQ], bf16, tag="wv")
            nc.gpsimd.dma_start(out=wv_sb[:], in_=wv[:])
            nc.gpsimd.dma_start(out=cos_sb[:, 512:S], in_=cosT[:, 512:S])
            nc.gpsimd.dma_start(out=sin_sb[:, 512:S], in_=sinT[:, 512:S])
            wo_sb = persist.tile([128, 2, D], bf16, tag="wo")
            nc.scalar.dma_start(out=wo_sb[:], in_=wo[:])
            for dc in range(4):
                nc.sync.dma_start(
                    out=xt_sb[dc][:, 1024:S],
                    in_=xT[128 * dc : 128 * (dc + 1), 1024:S],
                )
            for dc in range(4, N_DC):
                nc.scalar.dma_start(
                    out=xt_sb[dc][:, 1024:S],
                    in_=xT[128 * dc : 128 * (dc + 1), 1024:S],
                )

            # ---------------- persistent intermediates ----------------
            qT_sb = persist.tile([128, 2, S], bf16, tag="qT")  # [64h..., cc, s]
            kT_sb = persist.tile([128, 2, S], bf16, tag="kT")
            v_sb = persist.tile([128, N_KB, H_PER_CORE, HD + 1], bf16, tag="v")
            nc.vector.memset(v_sb[:, :, :, HD : HD + 1], 1.0)
            ctxT_sb = persist.tile([128, 2, S], bf16, tag="ctxT")  # unnormalized
            recip_dram = dram_pool.tile([N_SB, H_PER_CORE, 512], bf16, tag="rdram")
            ones_sb = persist.tile([1, 128], bf16, tag="ones")
            nc.gpsimd.memset(ones_sb[:], 1.0)

            # ---------------- pump machinery ----------------
            # pend_fast: slot-critical copies of the just-finished strip.
            # pend: FIFO of (tag, pe_ns, vec_ns, fn) chunks of independent work.
            pend_fast = deque()
            pend = deque()
            done_tags = set()

            def enq(pe, vec, fn, tag=None):
                pend.append((tag, pe, vec, fn))

            def _emit(item):
                t, _pe, _vec, fn = item
                fn()
                if t is not None:
                    done_tags.add(t)

            def drain_fast():
                while pend_fast:
                    _emit(pend_fast.popleft())

            def drain_until(tag):
                drain_fast()
                while tag not in done_tags:
                    assert pend, f"drain_until: tag {tag} not queued"
                    _emit(pend.popleft())

            def drain_all():
                drain_fast()
                while pend:
                    _emit(pend.popleft())

            def pump(pe_budget, vec_budget, force_fast=False):
                pe_s = vec_s = 0.0
                if force_fast and pend_fast:
                    # slot-critical copy of the previous strip: emit one per
                    # iteration regardless of budget so its PSUM slot frees
                    # before this strip's first AV matmuls
                    _emit(pend_fast.popleft())
                while pend_fast or pend:
                    q = pend_fast if pend_fast else pend
                    _t, pe, vec, fn = q[0]
                    if vec_s + vec > vec_budget:
                        break
                    if pe > 0.0 and pe_s >= pe_budget:
                        break
                    _emit(q.popleft())
                    pe_s += pe
                    vec_s += vec

            # ---------------- chunk generators ----------------
            DC_ORDER = [0, 1, 4, 5, 2, 3, 6, 7]  # matches DMA arrival order

            def rope_chunks(hold, dst_sb, cc, sb):
                """dst = src*cos + rotate_half(src)*sin, fp32 in, bf16 out.

                Three ~0.55us vector chunks; the partition shift is 4 small
                SBUF->SBUF DMAs on the gpsimd ring (sin is pre-shifted on the
                host so the product happens at the SOURCE rows).
                """
                ss = slice(512 * sb, 512 * (sb + 1))

                def m1():
                    hold["t1"] = rope_pool.tile([128, 512], bf16, tag="ropeA", name="t1")
                    nc.vector.tensor_mul(hold["t1"][:], hold["ps"][:], cos_sb[:, ss])

                def m2():
                    t2p = rope_pool.tile([128, 512], bf16, tag="ropeQ", name="t2p")
                    nc.vector.tensor_mul(t2p[:], hold["ps"][:], sin_sb[:, ss])
                    rot = rope_pool.tile([128, 512], bf16, tag="ropeB", name="rot")
                    for quarter in range(4):
                        o = 32 * quarter
                        so = o + 32 if quarter % 2 == 0 else o - 32
                        nc.gpsimd.dma_start(
                            out=rot[o : o + 32, :], in_=t2p[so : so + 32, :]
                        )
                    hold["rot"] = rot

                def a3():
                    nc.vector.tensor_add(
                        dst_sb[:, cc, ss], hold["t1"][:], hold["rot"][:]
                    )

                return [(0.0, 600.0, m1), (0.0, 600.0, m2), (0.0, 600.0, a3)]

            def enq_pqk(cc, sb):
                """q+k projection of head-pair cc, seq block sb, as chunks."""
                ss = slice(512 * sb, 512 * (sb + 1))
                tails = []
                for w_sb, dst in ((wq_sb, qT_sb), (wk_sb, kT_sb)):
                    hold = {}

                    def mk(pair, w_sb=w_sb, hold=hold):
                        def fn():
                            if "ps" not in hold:
                                hold["ps"] = psC.tile(
                                    [128, 512], f32, tag="proj", name="qk_ps"
                                )
                            for dc in pair:
                                nc.tensor.matmul(
                                    hold["ps"][:],
                                    w_sb[:, dc, 128 * cc : 128 * (cc + 1)],
                                    xt_sb[dc][:, ss],
                                    start=(dc == DC_ORDER[0]),
                                    stop=(dc == DC_ORDER[-1]),
                                )

                        return fn

                    for i in range(4):
                        enq(2 * MM512, 0.0, mk(tuple(DC_ORDER[2 * i : 2 * i + 2])))
                    rc = rope_chunks(hold, dst, cc, sb)
                    enq(*rc[0])
                    enq(*rc[1])
                    tails.append(rc[2])
                # the two adds last, so each rope's shift-DMA latency is
                # covered by the other tensor's projection matmuls
                enq(*tails[0])
                enq(*tails[1], tag=("pqk", cc, sb))

            def enq_pv(sb):
                """v projection for the 4 seq chunks of block sb."""
                for sc in range(4 * sb, 4 * sb + 4):
                    hold = {}

                    def mk(dcs, sc=sc, hold=hold):
                        def fn():
                            if "ps" not in hold:
                                hold["ps"] = psC.tile(
                                    [128, DQ], f32, tag="proj", name="v_ps"
                                )
                            for dc in dcs:
                                nc.tensor.matmul(
                                    hold["ps"][:],
                                    xt_sb[dc][:, 128 * sc : 128 * (sc + 1)],
                                    wv_sb[:, dc, :],
                                    start=(dc == 0),
                                    stop=(dc == N_DC - 1),
                                )

                        return fn

                    def cp(sc=sc, hold=hold):
                        nc.vector.tensor_copy(
                            v_sb[:, sc, :, 0:HD],
                            hold["ps"][:].rearrange("p (h d) -> p h d", h=H_PER_CORE),
                        )

                    enq(4 * MM256, 0.0, mk(tuple(range(4))))
                    enq(4 * MM256, 350.0, mk(tuple(range(4, 8))))
                    enq(0.0, 350.0, cp, tag=("pv", sb) if sc == 4 * sb + 3 else None)

            def op_chunk(qb, oc, tail=False):
                qs = slice(512 * qb, 512 * (qb + 1))

                def fn():
                    # at the tail psB's attention banks are free: alternate
                    # pools so the matmul stream isn't paced by cast latency
                    pool = psB if (tail and oc % 2 == 1) else psC
                    ptag = "ctx" if (tail and oc % 2 == 1) else "proj"
                    y_ps = pool.tile([128, 512], f32, tag=ptag, name="y_ps")
                    for cci in range(2):
                        nc.tensor.matmul(
                            y_ps[:],
                            wo_sb[:, cci, 128 * oc : 128 * (oc + 1)],
                            ctxT_sb[:, cci, qs],
                            start=(cci == 0),
                            stop=(cci == 1),
                        )
                    y_sb = yout_pool.tile([128, 512], bf16, tag="y", name="y_sb")
                    if tail and oc % 2 == 1:
                        # scalar engine is idle after the last exp: use it for
                        # half the casts and the second HWDGE ring for stores
                        nc.scalar.copy(y_sb[:], y_ps[:])
                        eng = nc.scalar
                    else:
                        nc.vector.tensor_copy(y_sb[:], y_ps[:])
                        eng = nc.sync
                    eng.dma_start(out=yT[128 * oc : 128 * (oc + 1), qs], in_=y_sb[:])

                return (2 * MM512, 640.0, fn)

            def enq_op(qb):
                for oc in range(N_DC):
                    enq(*op_chunk(qb, oc))

            # ---------------- normalization ----------------
            def boundary(cc, qb, ctx, inline=False):
                """Stage denominators + evacuate ctx, then the reciprocal
                broadcast chain.  As pump chunks: the two slot-critical copy
                bundles go on the fast queue; the recip chain follows FIFO."""
                qs = slice(512 * qb, 512 * (qb + 1))
                hold = {}

                def cpy(h):
                    def fn():
                        if "stage" not in hold:
                            hold["stage"] = small_pool.tile(
                                [1, 1024], f32, tag="stage", name="stage"
                            )
                        nc.vector.tensor_copy(
                            ctxT_sb[64 * h : 64 * (h + 1), cc, qs], ctx[h][0:HD, :]
                        )
                        nc.vector.tensor_copy(
                            hold["stage"][0:1, 512 * h : 512 * (h + 1)],
                            ctx[h][HD : HD + 1, :],
                        )

                    return fn

                def n2():
                    # repartition [1,1024] -> [64,16]: reciprocal is an
                    # iterative-divide DVE op (~8 cyc/elem of free size)
                    den_q = small_pool.tile([64, 16], f32, tag="den_q", name="den_q")
                    nc.sync.dma_start(out=den_q[:], in_=hold["stage"][0:1, 0:1024])
                    rec_q = small_pool.tile([64, 16], bf16, tag="rec_q", name="rec_q")
                    with nc.allow_low_precision(
                        reason="bf16 softmax denom matches bf16 attn weights"
                    ):
                        nc.vector.reciprocal(rec_q[:], den_q[:])
                    hold["rec"] = rec_q

                def n3():
                    nc.sync.dma_start(
                        out=recip_dram[qb, 2 * cc : 2 * cc + 2, :], in_=hold["rec"][:]
                    )
                    bc_sb = small_pool.tile([128, 512], bf16, tag="bcast", name="bc_sb")
                    for h in range(2):
                        row = recip_dram[qb, 2 * cc + h, :]
                        bcast = bass.AP(
                            tensor=row.tensor,
                            offset=row.offset,
                            ap=[[0, 64]] + list(row.ap)[-1:],
                        )
                        nc.sync.dma_start(
                            out=bc_sb[64 * h : 64 * (h + 1), :], in_=bcast
                        )
                    hold["bc"] = bc_sb

                def n4():
                    nc.vector.tensor_mul(
                        ctxT_sb[:, cc, qs], ctxT_sb[:, cc, qs], hold["bc"][:]
                    )

                if inline:
                    # tail chain: DRAM-free broadcast (ones-matmul into PSUM),
                    # scalar engine for half the copies, scratch matmuls gated
                    # on the chain's own data to keep the HAM clock warm
                    hold["stage"] = small_pool.tile(
                        [1, 1024], f32, tag="stage", name="stage"
                    )
                    nc.vector.tensor_copy(ctxT_sb[0:64, cc, qs], ctx[0][0:HD, :])
                    nc.vector.tensor_copy(
                        hold["stage"][0:1, 0:512], ctx[0][HD : HD + 1, :]
                    )
                    nc.scalar.copy(ctxT_sb[64:128, cc, qs], ctx[1][0:HD, :])
                    nc.scalar.copy(
                        hold["stage"][0:1, 512:1024], ctx[1][HD : HD + 1, :]
                    )
                    warm = psA.tile([128, 2, 512], f32, tag="score", name="warm")
                    for wi in range(4):
                        nc.tensor.matmul(
                            warm[:, 0, :],
                            ctxT_sb[:, cc, 512 * qb : 512 * qb + 128],
                            ctxT_sb[:, cc, qs],
                            start=True,
                            stop=True,
                        )
                    n2()
                    recrow = small_pool.tile([1, 1024], bf16, tag="recrow")
                    nc.sync.dma_start(out=recrow[:], in_=hold["rec"][:])
                    for wi in range(4):
                        nc.tensor.matmul(
                            warm[:, 1, :],
                            recrow[0:1, 0:128],
                            recrow[0:1, 0:512],
                            start=True,
                            stop=True,
                        )
                    # broadcast each head's reciprocals to all partitions via
                    # a K=1 ones-matmul into its own full bank (DRAM-free)
                    bc = [
                        psC.tile([128, 512], f32, tag="proj", name="bc0"),
                        psB.tile([128, 512], f32, tag="ctx", name="bc1"),
                    ]
                    for h in range(2):
                        nc.tensor.matmul(
                            bc[h][:],
                            ones_sb[:],
                            recrow[0:1, 512 * h : 512 * (h + 1)],
                            start=True,
                            stop=True,
                        )
                    for h in range(2):
                        nc.vector.tensor_mul(
                            ctxT_sb[64 * h : 64 * (h + 1), cc, qs],
                            ctxT_sb[64 * h : 64 * (h + 1), cc, qs],
                            bc[h][64 * h : 64 * (h + 1), :],
                        )
                else:
                    pend_fast.append((None, 0.0, 1200.0, cpy(0)))
                    pend_fast.append((None, 0.0, 1200.0, cpy(1)))
                    enq(0.0, 300.0, n2)
                    enq(0.0, 0.0, n3)
                    enq(0.0, 600.0, n4)

            # ---------------- attention strip ----------------
            def exp_cost(off):
                return (2 * (512 - off) + 352) / 1.2

            def attention(cc, qb):
                """Causal attention for head pair cc, query block qb.

                Per k-block (descending): two concurrent score matmuls (head h
                in PE row-group h) into one [128,2,512] PSUM tile, one exp over
                both heads' live columns, band-mask on diagonal blocks, and the
                AV matmuls two iterations delayed so the exp latency is hidden.
                Independent PE work is pumped into the per-iteration slack."""
                qs0 = 512 * qb
                nkb = 4 * qb + 4
                ctx = [
                    psB.tile([HD + 1, 512], f32, tag="ctx", name=f"ctx{h}")
                    for h in range(2)
                ]
                pend_av = deque()
                state = {"first": True}

                def flush_av(last=False):
                    pkb, poff, p_t = pend_av.popleft()
                    for h in range(2):
                        nc.tensor.matmul(
                            ctx[h][:, poff:512],
                            v_sb[:, pkb, 2 * cc + h, :],
                            p_t[:, h, poff:512],
                            start=state["first"],
                            stop=last,
                        )
                    state["first"] = False
                    return poff

                for idx, kb in enumerate(range(nkb - 1, -1, -1)):
                    j = kb - 4 * qb
                    off = 128 * j if j > 0 else 0
                    s_ps = psA.tile([128, 2, 512], f32, tag="score", name="s_ps")
                    for h in range(2):
                        hp = slice(64 * h, 64 * (h + 1))
                        nc.tensor.matmul(
                            s_ps[:, h, off:512],
                            kT_sb[hp, cc, 128 * kb : 128 * (kb + 1)],
                            qT_sb[hp, cc, qs0 + off : qs0 + 512],
                            start=True,
                            stop=True,
                        )
                    a_t = attn_pool.tile([128, 2, 512], bf16, tag="attnT", name="a_t")
                    nc.scalar.activation(
                        a_t[:, :, off:512],
                        s_ps[:, :, off:512],
                        EXPF,
                        scale=float(1.0 / np.sqrt(HD)),
                    )
                    if j >= 0:
                        nc.vector.tensor_mul(
                            a_t[:, :, off : off + 128],
                            a_t[:, :, off : off + 128],
                            bmask_sb[:],
                        )
                    pend_av.append((kb, off, a_t))
                    pe_used = (512 - off) / 2.4 + 90.0
                    if idx >= 2:
                        poff = flush_av()
                        pe_used += 2 * (512 - poff) / 2.4 + 60.0
                    budget = exp_cost(off) - pe_used + PUMP_SLACK
                    vec_b = VEC_BUDGET - (200.0 if j >= 0 else 0.0)
                    pump(budget, vec_b, force_fast=(idx < 2))
                flush_av()
                flush_av(last=True)
                return ctx

            # ---------------- main schedule ----------------
            enq_pqk(0, 0)
            drain_all()  # first projection inline (nothing to overlap yet)
            enq_pv(0)
            enq_pqk(1, 0)
            enq_pqk(0, 1)

            drain_until(("pv", 0))
            ctx = attention(0, 0)
            boundary(0, 0, ctx)
            enq_pv(1)
            enq_pqk(1, 1)

            drain_until(("pqk", 1, 0))
            ctx = attention(1, 0)
            boundary(1, 0, ctx)
            enq_pqk(0, 2)

            drain_until(("pqk", 0, 1))
            drain_until(("pv", 1))
            ctx = attention(0, 1)
            boundary(0, 1, ctx)
            enq_pv(2)
            enq_pqk(1, 2)

            drain_until(("pqk", 1, 1))
            ctx = attention(1, 1)
            boundary(1, 1, ctx)
            enq_op(0)
            enq_pqk(0, 3)

            drain_until(("pqk", 0, 2))
            drain_until(("pv", 2))
            ctx = attention(0, 2)
            boundary(0, 2, ctx)
            enq_pv(3)
            enq_pqk(1, 3)

            drain_until(("pqk", 1, 2))
            ctx = attention(1, 2)
            boundary(1, 2, ctx)
            enq_op(1)

            drain_until(("pqk", 0, 3))
            drain_until(("pv", 3))
            ctx = attention(0, 3)
            boundary(0, 3, ctx)
            enq_op(2)

            drain_until(("pqk", 1, 3))
            ctx = attention(1, 3)
            drain_all()
            boundary(1, 3, ctx, inline=True)
            for oc in range(N_DC):
                op_chunk(3, oc, tail=True)[2]()

    nc.compile()
    return nc


def _rope_tables():
    inv_freq = (
        1.0 / (THETA ** (np.arange(0, HD, 2, dtype=np.float32) / HD))
    ).astype(np.float32)
    pos = np.arange(S, dtype=np.float32)
    ang = pos[:, None] * inv_freq[None, :]  # [S, 32]
    cos_half = np.cos(ang).astype(np.float32).T  # [32, S]
    sin_half = np.sin(ang).astype(np.float32).T
    # per-head 64 rows: cos rows duplicated. The sin table is PRE-SHIFTED:
    # row p holds sin_signed[partner(p)] (partner = rotate-half swap), so the
    # kernel multiplies at the source rows and a plain partition-shift DMA
    # finishes rotate-half: sinx per head = (+sin | -sin).
    cos64 = np.concatenate([cos_half, cos_half], axis=0)
    sinx64 = np.concatenate([sin_half, -sin_half], axis=0)
    cosT = np.concatenate([cos64, cos64], axis=0)  # [128, S] two heads
    sinT = np.concatenate([sinx64, sinx64], axis=0)
    bf = ml_dtypes.bfloat16
    return (
        np.ascontiguousarray(cosT).astype(bf),
        np.ascontiguousarray(sinT).astype(bf),
    )


def _bmask():
    k = np.arange(128)[:, None]
    t = np.arange(128)[None, :]
    blk = (k <= t).astype(ml_dtypes.bfloat16)
    return np.ascontiguousarray(np.concatenate([blk, blk], axis=1))


def _swizzle(w):
    # [128c+p, n] -> [p, c*ncol + n]: contiguous per-partition DMA layout
    nrow, ncol = w.shape
    c = nrow // 128
    return np.ascontiguousarray(
        w.reshape(c, 128, ncol).transpose(1, 0, 2).reshape(128, c * ncol)
    )


def kernel(x, W_q, W_k, W_v, W_o):
    global _CACHED
    from concourse.bass_utils import run_bass_kernel_spmd

    if _CACHED is None:
        _CACHED = _build_kernel()
    nc = _CACHED

    bf = ml_dtypes.bfloat16
    cosT, sinT = _rope_tables()
    bmask = _bmask()
    x = np.asarray(x)
    W_q, W_k, W_v, W_o = (np.asarray(w) for w in (W_q, W_k, W_v, W_o))
    xT = [np.ascontiguousarray(x[b].T).astype(bf) for b in range(B)]

    in_maps = []
    for c in range(N_CORES):
        b, g = divmod(c, 4)
        cols = slice(DQ * g, DQ * (g + 1))
        in_maps.append(
            {
                "xT": xT[b],
                "wq": _swizzle(W_q[:, cols].astype(bf)),
                "wk": _swizzle(W_k[:, cols].astype(bf)),
                "wv": _swizzle(W_v[:, cols].astype(bf)),
                "wo": _swizzle(W_o[cols, :].astype(bf)),
                "cosT": cosT,
                "sinT": sinT,
                "bmask": bmask,
            }
        )

    res = run_bass_kernel_spmd(nc, in_maps, core_ids=list(range(N_CORES)))
    kernel.last_results = res

    y = np.empty((B, S, D), dtype=np.float32)
    for b in range(B):
        acc = res.results[4 * b]["yT"].astype(np.float32)
        for g in range(1, 4):
            acc += res.results[4 * b + g]["yT"].astype(np.float32)
        y[b] = acc.T
    return y


# revision 24
# speedup vs baseline: 1.2055x; 1.0257x over previous
"""Multi-head attention (RoPE, causal) Trainium2 kernel, SPMD over 8 NeuronCores.

Problem: x[2,2048,1024] @ {W_q,W_k,W_v}[1024,1024] -> 16-head causal attention
with RoPE -> @ W_o[1024,1024].

Sharding (batch x heads): core c handles batch b=c//4 and head group g=c%4
(4 heads = 256 of the 1024 qkv dims). Each core computes its heads' QKV
projections, RoPE, causal attention, and a partial out-projection
(ctx_g @ W_o[256g:256g+256, :]). The host sums the 4 partials per batch
(unshard of a partial-sum sharding) and transposes back.

On-device layout is fully transposed ([feature, seq]): scoresT[k,q] = K^T.T @
Q^T, the softmax denominator falls out of the AV matmul via a ones-column
appended to V, and the out-projection consumes ctxT directly.

Schedule: the exp of the attention weights runs only on the Scalar(ACT)
engine (~1 elem/lane/cycle @1.2GHz) and totals ~80us; total PE matmul work
is ~100us. The kernel therefore runs one long software-pipelined stream of
attention k-block steps (the scalar backbone) and PUMPS independent PE work
(QKV projections, out-projections, normalize chains of finished strips)
into the per-kb slack so both engines stay busy concurrently. Causal
structure is exploited at 128-column granularity: for diagonal key-blocks
only columns >= the block offset are computed in scores/exp/AV, and a single
[128,2,128] band mask handles the ragged diagonal. k-blocks run in
DESCENDING order so the final AV (full width) carries the stop flag while
the first (narrowest) carries start (start clears the whole PSUM bank, so
partial-width accumulation stays correct).
"""

import numpy as np
import ml_dtypes
from collections import deque

B = 2
S = 2048
D = 1024
H = 16
HD = 64
N_CORES = 8
H_PER_CORE = 4
DQ = H_PER_CORE * HD  # 256 qkv dims per core
N_DC = D // 128  # 8 contraction chunks
N_SB = S // 512  # 4 seq blocks of 512 (query blocks)
N_KB = S // 128  # 16 key blocks of 128
THETA = 10000.0

# ---- schedule tuning knobs (ns estimates for the pump budget model) ----
N_WARM = 5          # throwaway matmuls to lift the HAM clock gate at t=0
AV_DEPTH = 4        # AV matmuls trail their scores by this many kb-steps
PUMP_SLACK = 120.0  # extra per-iteration PE budget beyond the exp/PE gap
VEC_BUDGET = 650.0  # per-iteration vector-work ceiling for pumped chunks
MM512 = 225.0       # warm N=512 matmul estimate
MM256 = 120.0

_CACHED = None


def _build_kernel():
    import concourse.bass as bass
    import concourse.mybir as mybir
    import concourse.tile as tile
    from concourse import bacc

    f32 = mybir.dt.float32
    bf16 = mybir.dt.bfloat16
    EXPF = mybir.ActivationFunctionType.Exp

    nc = bacc.Bacc(None, target_bir_lowering=False, num_devices=N_CORES)

    xT = nc.dram_tensor("xT", [D, S], bf16, kind="ExternalInput")
    # weights are pre-swizzled on the host to [partition, chunk*cols] so the
    # input DMAs are fully contiguous per partition (512B descriptors from a
    # rearrange on the device side measured ~3x slower)
    wq = nc.dram_tensor("wq", [128, N_DC * DQ], bf16, kind="ExternalInput")
    wk = nc.dram_tensor("wk", [128, N_DC * DQ], bf16, kind="ExternalInput")
    wv = nc.dram_tensor("wv", [128, N_DC * DQ], bf16, kind="ExternalInput")
    wo = nc.dram_tensor("wo", [128, 2 * D], bf16, kind="ExternalInput")
    cosT = nc.dram_tensor("cosT", [128, S], bf16, kind="ExternalInput")
    sinT = nc.dram_tensor("sinT", [128, S], bf16, kind="ExternalInput")
    # bmask[k, 128h + t] = 1.0 if k <= t else 0 (ragged diagonal band mask)
    bmask = nc.dram_tensor("bmask", [128, 2 * 128], bf16, kind="ExternalInput")
    yT = nc.dram_tensor("yT", [D, S], bf16, kind="ExternalOutput")

    with tile.TileContext(nc) as tc:
        with (
            tc.tile_pool(name="persist", bufs=1) as persist,
            tc.tile_pool(name="attn", bufs=8) as attn_pool,
            tc.tile_pool(name="rope", bufs=4) as rope_pool,
            tc.tile_pool(name="small", bufs=4) as small_pool,
            tc.tile_pool(name="yout", bufs=4) as yout_pool,
            tc.tile_pool(name="dram", bufs=1, space="DRAM") as dram_pool,
            tc.tile_pool(name="psA", bufs=2, space="PSUM") as psA,  # scores
            tc.tile_pool(name="psB", bufs=2, space="PSUM") as psB,  # ctx accum
            tc.tile_pool(name="psC", bufs=2, space="PSUM") as psC,  # proj/y
        ):
            # ---------------- PE warm-up + ACT table preload ----------------
            # The HAM clock gate needs ~3.4us of PE activity for 2.4GHz; run
            # throwaway matmuls on a memset tile from t~0.  A tiny dummy exp
            # forces the ~2.7us exp table-set load before the first real exp.
            warm_sb = persist.tile([128, 512], bf16, tag="warm")
            nc.gpsimd.memset(warm_sb[:], 0.125)
            dummy_e = persist.tile([1, 8], bf16, tag="dummy")
            nc.scalar.activation(dummy_e[:], warm_sb[0:1, 0:8], EXPF, scale=1.0)
            for wi in range(N_WARM):
                wps = psC.tile([128, 512], f32, tag="proj", name="warm")
                nc.tensor.matmul(
                    wps[:], warm_sb[:, 0:128], warm_sb[:], start=True, stop=True
                )

            # ---------------- input DMA ----------------
            # Three parallel issue paths (two HWDGE rings + gpsimd SWDGE),
            # ordered by first-use deadline.  xt is loaded in column halves
            # and the weights in chunk halves so the first projection matmuls
            # only wait on a few hundred KB per ring.
            wq_sb = persist.tile([128, N_DC, DQ], bf16, tag="wq")
            wk_sb = persist.tile([128, N_DC, DQ], bf16, tag="wk")
            xt_sb = [
                persist.tile([128, S], bf16, tag=f"xt{dc}", name=f"xt{dc}")
                for dc in range(N_DC)
            ]
            # first-needed quarters in small chunks so the first projection
            # matmuls start ~2us earlier; the rest in bigger chunks
            nc.sync.dma_start(out=wq_sb[:, 0:2, :], in_=wq[:, 0 : 2 * DQ])
            nc.scalar.dma_start(out=wk_sb[:, 0:2, :], in_=wk[:, 0 : 2 * DQ])
            for dc in range(2):
                nc.sync.dma_start(
                    out=xt_sb[dc][:, 0:512],
                    in_=xT[128 * dc : 128 * (dc + 1), 0:512],
                )
            for dc in range(4, 6):
                nc.scalar.dma_start(
                    out=xt_sb[dc][:, 0:512],
                    in_=xT[128 * dc : 128 * (dc + 1), 0:512],
                )
            nc.sync.dma_start(out=wq_sb[:, 2:8, :], in_=wq[:, 2 * DQ : 8 * DQ])
            nc.scalar.dma_start(out=wk_sb[:, 2:8, :], in_=wk[:, 2 * DQ : 8 * DQ])
            for dc in range(2, 4):
                nc.sync.dma_start(
                    out=xt_sb[dc][:, 0:512],
                    in_=xT[128 * dc : 128 * (dc + 1), 0:512],
                )
            for dc in range(6, 8):
                nc.scalar.dma_start(
                    out=xt_sb[dc][:, 0:512],
                    in_=xT[128 * dc : 128 * (dc + 1), 0:512],
                )
            for dc in range(4):
                nc.sync.dma_start(
                    out=xt_sb[dc][:, 512:1024],
                    in_=xT[128 * dc : 128 * (dc + 1), 512:1024],
                )
            for dc in range(4, 8):
                nc.scalar.dma_start(
                    out=xt_sb[dc][:, 512:1024],
                    in_=xT[128 * dc : 128 * (dc + 1), 512:1024],
                )
            cos_sb = persist.tile([128, S], bf16, tag="cos")
            sin_sb = persist.tile([128, S], bf16, tag="sin")
            nc.gpsimd.dma_start(out=cos_sb[:, 0:512], in_=cosT[:, 0:512])
            nc.gpsimd.dma_start(out=sin_sb[:, 0:512], in_=sinT[:, 0:512])
            bmask_sb = persist.tile([128, 2, 128], bf16, tag="bmask")
            nc.gpsimd.dma_start(
                out=bmask_sb[:], in_=bmask.rearrange("p (h t) -> p h t", h=2)
            )
            wv_sb = persist.tile([128, N_DC, DQ], bf16, tag="wv")
            nc.gpsimd.dma_start(out=wv_sb[:], in_=wv[:])
            nc.gpsimd.dma_start(out=cos_sb[:, 512:S], in_=cosT[:, 512:S])
            nc.gpsimd.dma_start(out=sin_sb[:, 512:S], in_=sinT[:, 512:S])
            wo_sb = persist.tile([128, 2, D], bf16, tag="wo")
            nc.scalar.dma_start(out=wo_sb[:], in_=wo[:])
            for dc in range(4):
                nc.sync.dma_start(
                    out=xt_sb[dc][:, 1024:S],
                    in_=xT[128 * dc : 128 * (dc + 1), 1024:S],
                )
            for dc in range(4, N_DC):
                nc.scalar.dma_start(
                    out=xt_sb[dc][:, 1024:S],
                    in_=xT[128 * dc : 128 * (dc + 1), 1024:S],
                )

            # ---------------- persistent intermediates ----------------
            qT_sb = persist.tile([128, 2, S], bf16, tag="qT")  # [64h..., cc, s]
            kT_sb = persist.tile([128, 2, S], bf16, tag="kT")
            v_sb = persist.tile([128, N_KB, H_PER_CORE, HD + 1], bf16, tag="v")
            nc.vector.memset(v_sb[:, :, :, HD : HD + 1], 1.0)
            ctxT_sb = persist.tile([128, 2, S], bf16, tag="ctxT")  # unnormalized
            recip_dram = dram_pool.tile([N_SB, H_PER_CORE, 512], bf16, tag="rdram")
            ones_sb = persist.tile([1, 128], bf16, tag="ones")
            nc.gpsimd.memset(ones_sb[:], 1.0)

            # ---------------- pump machinery ----------------
            # pend_fast: slot-critical copies of the just-finished strip.
            # pend: FIFO of (tag, pe_ns, vec_ns, fn) chunks of independent work.
            pend_fast = deque()
            pend = deque()
            done_tags = set()

            def enq(pe, vec, fn, tag=None):
                pend.append((tag, pe, vec, fn))

            def _emit(item):
                t, _pe, _vec, fn = item
                fn()
                if t is not None:
                    done_tags.add(t)

            def drain_fast():
                while pend_fast:
                    _emit(pend_fast.popleft())

            def drain_until(tag):
                drain_fast()
                while tag not in done_tags:
                    assert pend, f"drain_until: tag {tag} not queued"
                    _emit(pend.popleft())

            def drain_all():
                drain_fast()
                while pend:
                    _emit(pend.popleft())

            def pump(pe_budget, vec_budget):
                pe_s = vec_s = 0.0
                # slot-critical copies of a just-finished strip: emit ahead of
                # budget so their PSUM slots free before the next strip's AVs
                for _ in range(2):
                    if pend_fast:
                        _emit(pend_fast.popleft())
                while pend_fast or pend:
                    q = pend_fast if pend_fast else pend
                    _t, pe, vec, fn = q[0]
                    if vec_s + vec > vec_budget:
                        break
                    if pe > 0.0 and pe_s >= pe_budget:
                        break
                    _emit(q.popleft())
                    pe_s += pe
                    vec_s += vec

            # ---------------- chunk generators ----------------
            DC_ORDER = [0, 1, 4, 5, 2, 3, 6, 7]  # matches DMA arrival order

            def rope_chunks(hold, dst_sb, cc, sb):
                """dst = src*cos + rotate_half(src)*sin, fp32 in, bf16 out.

                Three ~0.55us vector chunks; the partition shift is 4 small
                SBUF->SBUF DMAs on the gpsimd ring (sin is pre-shifted on the
                host so the product happens at the SOURCE rows).
                """
                ss = slice(512 * sb, 512 * (sb + 1))

                def m1():
                    hold["t1"] = rope_pool.tile([128, 512], bf16, tag="ropeA", name="t1")
                    nc.vector.tensor_mul(hold["t1"][:], hold["ps"][:], cos_sb[:, ss])

                def m2():
                    t2p = rope_pool.tile([128, 512], bf16, tag="ropeQ", name="t2p")
                    nc.vector.tensor_mul(t2p[:], hold["ps"][:], sin_sb[:, ss])
                    rot = rope_pool.tile([128, 512], bf16, tag="ropeB", name="rot")
                    for quarter in range(4):
                        o = 32 * quarter
                        so = o + 32 if quarter % 2 == 0 else o - 32
                        nc.gpsimd.dma_start(
                            out=rot[o : o + 32, :], in_=t2p[so : so + 32, :]
                        )
                    hold["rot"] = rot

                def a3():
                    nc.vector.tensor_add(
                        dst_sb[:, cc, ss], hold["t1"][:], hold["rot"][:]
                    )

                return [(0.0, 600.0, m1), (0.0, 600.0, m2), (0.0, 600.0, a3)]

            def enq_pqk(cc, sb):
                """q+k projection of head-pair cc, seq block sb, as chunks."""
                ss = slice(512 * sb, 512 * (sb + 1))
                tails = []
                for w_sb, dst in ((wq_sb, qT_sb), (wk_sb, kT_sb)):
                    hold = {}

                    def mk(pair, w_sb=w_sb, hold=hold):
                        def fn():
                            if "ps" not in hold:
                                hold["ps"] = psC.tile(
                                    [128, 512], f32, tag="proj", name="qk_ps"
                                )
                            for dc in pair:
                                nc.tensor.matmul(
                                    hold["ps"][:],
                                    w_sb[:, dc, 128 * cc : 128 * (cc + 1)],
                                    xt_sb[dc][:, ss],
                                    start=(dc == DC_ORDER[0]),
                                    stop=(dc == DC_ORDER[-1]),
                                )

                        return fn

                    for i in range(4):
                        enq(2 * MM512, 0.0, mk(tuple(DC_ORDER[2 * i : 2 * i + 2])))
                    rc = rope_chunks(hold, dst, cc, sb)
                    enq(*rc[0])
                    enq(*rc[1])
                    tails.append(rc[2])
                # the two adds last, so each rope's shift-DMA latency is
                # covered by the other tensor's projection matmuls
                enq(*tails[0])
                enq(*tails[1], tag=("pqk", cc, sb))

            def enq_pv(sb):
                """v projection for the 4 seq chunks of block sb."""
                for sc in range(4 * sb, 4 * sb + 4):
                    hold = {}

                    def mk(dcs, sc=sc, hold=hold):
                        def fn():
                            if "ps" not in hold:
                                hold["ps"] = psC.tile(
                                    [128, DQ], f32, tag="proj", name="v_ps"
                                )
                            for dc in dcs:
                                nc.tensor.matmul(
                                    hold["ps"][:],
                                    xt_sb[dc][:, 128 * sc : 128 * (sc + 1)],
                                    wv_sb[:, dc, :],
                                    start=(dc == 0),
                                    stop=(dc == N_DC - 1),
                                )

                        return fn

                    def cp(sc=sc, hold=hold):
                        nc.vector.tensor_copy(
                            v_sb[:, sc, :, 0:HD],
                            hold["ps"][:].rearrange("p (h d) -> p h d", h=H_PER_CORE),
                        )

                    enq(4 * MM256, 0.0, mk(tuple(range(4))))
                    enq(4 * MM256, 350.0, mk(tuple(range(4, 8))))
                    enq(0.0, 350.0, cp, tag=("pv", sb) if sc == 4 * sb + 3 else None)

            def op_chunk(qb, oc, tail=False):
                qs = slice(512 * qb, 512 * (qb + 1))

                def fn():
                    # at the tail psB's attention banks are free: alternate
                    # pools so the matmul stream isn't paced by cast latency
                    pool = psB if (tail and oc % 2 == 1) else psC
                    ptag = "ctx" if (tail and oc % 2 == 1) else "proj"
                    y_ps = pool.tile([128, 512], f32, tag=ptag, name="y_ps")
                    for cci in range(2):
                        nc.tensor.matmul(
                            y_ps[:],
                            wo_sb[:, cci, 128 * oc : 128 * (oc + 1)],
                            ctxT_sb[:, cci, qs],
                            start=(cci == 0),
                            stop=(cci == 1),
                        )
                    y_sb = yout_pool.tile([128, 512], bf16, tag="y", name="y_sb")
                    if tail and oc % 2 == 1:
                        # scalar engine is idle after the last exp: use it for
                        # half the casts and the second HWDGE ring for stores
                        nc.scalar.copy(y_sb[:], y_ps[:])
                        eng = nc.scalar
                    else:
                        nc.vector.tensor_copy(y_sb[:], y_ps[:])
                        eng = nc.sync
                    eng.dma_start(out=yT[128 * oc : 128 * (oc + 1), qs], in_=y_sb[:])

                return (2 * MM512, 640.0, fn)

            def enq_op(qb):
                for oc in range(N_DC):
                    enq(*op_chunk(qb, oc))

            # ---------------- normalization ----------------
            def boundary(cc, qb, ctx, inline=False):
                """Stage denominators + evacuate ctx, then the reciprocal
                broadcast chain.  As pump chunks: the two slot-critical copy
                bundles go on the fast queue; the recip chain follows FIFO."""
                qs = slice(512 * qb, 512 * (qb + 1))
                hold = {}

                def cpy(h):
                    def fn():
                        if "stage" not in hold:
                            hold["stage"] = small_pool.tile(
                                [1, 1024], f32, tag="stage", name="stage"
                            )
                        nc.vector.tensor_copy(
                            ctxT_sb[64 * h : 64 * (h + 1), cc, qs], ctx[h][0:HD, :]
                        )
                        nc.vector.tensor_copy(
                            hold["stage"][0:1, 512 * h : 512 * (h + 1)],
                            ctx[h][HD : HD + 1, :],
                        )

                    return fn

                def n2():
                    # repartition [1,1024] -> [64,16]: reciprocal is an
                    # iterative-divide DVE op (~8 cyc/elem of free size)
                    den_q = small_pool.tile([64, 16], f32, tag="den_q", name="den_q")
                    nc.sync.dma_start(out=den_q[:], in_=hold["stage"][0:1, 0:1024])
                    rec_q = small_pool.tile([64, 16], bf16, tag="rec_q", name="rec_q")
                    with nc.allow_low_precision(
                        reason="bf16 softmax denom matches bf16 attn weights"
                    ):
                        nc.vector.reciprocal(rec_q[:], den_q[:])
                    hold["rec"] = rec_q

                def n3():
                    nc.sync.dma_start(
                        out=recip_dram[qb, 2 * cc : 2 * cc + 2, :], in_=hold["rec"][:]
                    )
                    bc_sb = small_pool.tile([128, 512], bf16, tag="bcast", name="bc_sb")
                    for h in range(2):
                        row = recip_dram[qb, 2 * cc + h, :]
                        bcast = bass.AP(
                            tensor=row.tensor,
                            offset=row.offset,
                            ap=[[0, 64]] + list(row.ap)[-1:],
                        )
                        nc.sync.dma_start(
                            out=bc_sb[64 * h : 64 * (h + 1), :], in_=bcast
                        )
                    hold["bc"] = bc_sb

                def n4():
                    nc.vector.tensor_mul(
                        ctxT_sb[:, cc, qs], ctxT_sb[:, cc, qs], hold["bc"][:]
                    )

                if inline:
                    # tail chain: denominator staging first (it gates the
                    # reciprocal chain), all on the vector engine (scalar is
                    # still draining the last exps), then a DRAM-free
                    # broadcast via ones-matmuls into PSUM; scratch matmuls
                    # gated on the chain's own data keep the HAM clock warm
                    hold["stage"] = small_pool.tile(
                        [1, 1024], f32, tag="stage", name="stage"
                    )
                    nc.vector.tensor_copy(
                        hold["stage"][0:1, 0:512], ctx[0][HD : HD + 1, :]
                    )
                    nc.vector.tensor_copy(
                        hold["stage"][0:1, 512:1024], ctx[1][HD : HD + 1, :]
                    )
                    n2()
                    nc.vector.tensor_copy(ctxT_sb[0:64, cc, qs], ctx[0][0:HD, :])
                    nc.vector.tensor_copy(ctxT_sb[64:128, cc, qs], ctx[1][0:HD, :])
                    warm = psA.tile([128, 2, 512], f32, tag="score", name="warm")
                    for wi in range(4):
                        nc.tensor.matmul(
                            warm[:, 0, :],
                            ctxT_sb[:, cc, 512 * qb : 512 * qb + 128],
                            ctxT_sb[:, cc, qs],
                            start=True,
                            stop=True,
                        )
                    recrow = small_pool.tile([1, 1024], bf16, tag="recrow")
                    nc.sync.dma_start(out=recrow[:], in_=hold["rec"][:])
                    for wi in range(4):
                        nc.tensor.matmul(
                            warm[:, 1, :],
                            recrow[0:1, 0:128],
                            recrow[0:1, 0:512],
                            start=True,
                            stop=True,
                        )
                    # broadcast each head's reciprocals to all partitions via
                    # a K=1 ones-matmul into its own full bank (DRAM-free)
                    bc = [
                        psC.tile([128, 512], f32, tag="proj", name="bc0"),
                        psB.tile([128, 512], f32, tag="ctx", name="bc1"),
                    ]
                    for h in range(2):
                        nc.tensor.matmul(
                            bc[h][:],
                            ones_sb[:],
                            recrow[0:1, 512 * h : 512 * (h + 1)],
                            start=True,
                            stop=True,
                        )
                    for h in range(2):
                        nc.vector.tensor_mul(
                            ctxT_sb[64 * h : 64 * (h + 1), cc, qs],
                            ctxT_sb[64 * h : 64 * (h + 1), cc, qs],
                            bc[h][64 * h : 64 * (h + 1), :],
                        )
                else:
                    pend_fast.append((None, 0.0, 1200.0, cpy(0)))
                    pend_fast.append((None, 0.0, 1200.0, cpy(1)))
                    enq(0.0, 300.0, n2)
                    enq(0.0, 0.0, n3)
                    enq(0.0, 600.0, n4)

            # ---------------- attention strip ----------------
            def exp_cost(off):
                return (2 * (512 - off) + 352) / 1.2

            # AV pipeline state: spans strip boundaries so the PE never waits
            # for a strip's last exps before starting the next strip's scores
            pend_av = deque()

            def flush_av():
                e = pend_av.popleft()
                for h in range(2):
                    nc.tensor.matmul(
                        e["ctx"][h][:, e["off"] : 512],
                        v_sb[:, e["kb"], 2 * e["cc"] + h, :],
                        e["a_t"][:, h, e["off"] : 512],
                        start=e["first"],
                        stop=e["last"],
                    )
                if e["last"] and e["on_done"] is not None:
                    e["on_done"](e["strip_ctx"])
                return e["off"]

            def attention(cc, qb, on_done):
                """Causal attention for head pair cc, query block qb.

                Per k-block (descending): two concurrent score matmuls (head h
                in PE row-group h) into one [128,2,512] PSUM tile, one exp over
                both heads' live columns, band-mask on diagonal blocks, and the
                AV matmuls AV_DEPTH kb-steps delayed (carrying across strip
                boundaries) so the exp latency is always hidden.  Independent
                PE work is pumped into the per-iteration slack.  on_done(ctx)
                fires right after the strip's last AV is emitted (it enqueues
                the normalization chain)."""
                qs0 = 512 * qb
                nkb = 4 * qb + 4
                ctx = [
                    psB.tile([HD + 1, 512], f32, tag="ctx", name=f"ctx{h}")
                    for h in range(2)
                ]
                for idx, kb in enumerate(range(nkb - 1, -1, -1)):
                    j = kb - 4 * qb
                    off = 128 * j if j > 0 else 0
                    s_ps = psA.tile([128, 2, 512], f32, tag="score", name="s_ps")
                    for h in range(2):
                        hp = slice(64 * h, 64 * (h + 1))
                        nc.tensor.matmul(
                            s_ps[:, h, off:512],
                            kT_sb[hp, cc, 128 * kb : 128 * (kb + 1)],
                            qT_sb[hp, cc, qs0 + off : qs0 + 512],
                            start=True,
                            stop=True,
                        )
                    a_t = attn_pool.tile([128, 2, 512], bf16, tag="attnT", name="a_t")
                    nc.scalar.activation(
                        a_t[:, :, off:512],
                        s_ps[:, :, off:512],
                        EXPF,
                        scale=float(1.0 / np.sqrt(HD)),
                    )
                    if j >= 0:
                        nc.vector.tensor_mul(
                            a_t[:, :, off : off + 128],
                            a_t[:, :, off : off + 128],
                            bmask_sb[:],
                        )
                    pend_av.append(
                        dict(
                            ctx=ctx,
                            strip_ctx=ctx,
                            cc=cc,
                            kb=kb,
                            off=off,
                            a_t=a_t,
                            first=(idx == 0),
                            last=(kb == 0),
                            on_done=on_done,
                        )
                    )
                    pe_used = (512 - off) / 2.4 + 90.0
                    will_flush = len(pend_av) >= AV_DEPTH
                    if will_flush:
                        pe_used += 2 * (512 - pend_av[0]["off"]) / 2.4 + 60.0
                    budget = exp_cost(off) - pe_used + PUMP_SLACK
                    vec_b = VEC_BUDGET - (200.0 if j >= 0 else 0.0)
                    # pump before flushing: if the head AV opens a new strip
                    # whose PSUM slot is still draining, the PE chews pumped
                    # work instead of head-of-line blocking on the slot
                    pump(budget, vec_b)
                    if will_flush:
                        flush_av()
                return ctx

            # ---------------- main schedule ----------------
            enq_pqk(0, 0)
            drain_all()  # first projection inline (nothing to overlap yet)
            enq_pv(0)
            enq_pqk(1, 0)
            enq_pqk(0, 1)

            def bnd(cc, qb):
                return lambda c: boundary(cc, qb, c)

            drain_until(("pv", 0))
            attention(0, 0, bnd(0, 0))
            enq_pv(1)
            enq_pqk(1, 1)

            drain_until(("pqk", 1, 0))
            attention(1, 0, bnd(1, 0))
            enq_pqk(0, 2)

            drain_until(("pqk", 0, 1))
            drain_until(("pv", 1))
            attention(0, 1, bnd(0, 1))
            enq_pv(2)
            enq_pqk(1, 2)

            drain_until(("pqk", 1, 1))
            attention(1, 1, bnd(1, 1))
            enq_op(0)
            enq_pqk(0, 3)

            drain_until(("pqk", 0, 2))
            drain_until(("pv", 2))
            attention(0, 2, bnd(0, 2))
            enq_pv(3)
            enq_pqk(1, 3)

            drain_until(("pqk", 1, 2))
            attention(1, 2, bnd(1, 2))
            enq_op(1)

            drain_until(("pqk", 0, 3))
            drain_until(("pv", 3))
            attention(0, 3, bnd(0, 3))
            enq_op(2)

            drain_until(("pqk", 1, 3))
            ctx = attention(1, 3, None)
            drain_all()
            while pend_av:
                flush_av()
            boundary(1, 3, ctx, inline=True)
            for oc in range(N_DC):
                op_chunk(3, oc, tail=True)[2]()

    nc.compile()
    return nc


def _rope_tables():
    inv_freq = (
        1.0 / (THETA ** (np.arange(0, HD, 2, dtype=np.float32) / HD))
    ).astype(np.float32)
    pos = np.arange(S, dtype=np.float32)
    ang = pos[:, None] * inv_freq[None, :]  # [S, 32]
    cos_half = np.cos(ang).astype(np.float32).T  # [32, S]
    sin_half = np.sin(ang).astype(np.float32).T
    # per-head 64 rows: cos rows duplicated. The sin table is PRE-SHIFTED:
    # row p holds sin_signed[partner(p)] (partner = rotate-half swap), so the
    # kernel multiplies at the source rows and a plain partition-shift DMA
    # finishes rotate-half: sinx per head = (+sin | -sin).
    cos64 = np.concatenate([cos_half, cos_half], axis=0)
    sinx64 = np.concatenate([sin_half, -sin_half], axis=0)
    cosT = np.concatenate([cos64, cos64], axis=0)  # [128, S] two heads
    sinT = np.concatenate([sinx64, sinx64], axis=0)
    bf = ml_dtypes.bfloat16
    return (
        np.ascontiguousarray(cosT).astype(bf),
        np.ascontiguousarray(sinT).astype(bf),
    )


def _bmask():
    k = np.arange(128)[:, None]
    t = np.arange(128)[None, :]
    blk = (k <= t).astype(ml_dtypes.bfloat16)
    return np.ascontiguousarray(np.concatenate([blk, blk], axis=1))


def _swizzle(w):
    # [128c+p, n] -> [p, c*ncol + n]: contiguous per-partition DMA layout
    nrow, ncol = w.shape
    c = nrow // 128
    return np.ascontiguousarray(
        w.reshape(c, 128, ncol).transpose(1, 0, 2).reshape(128, c * ncol)
    )


def kernel(x, W_q, W_k, W_v, W_o):
    global _CACHED
    from concourse.bass_utils import run_bass_kernel_spmd

    if _CACHED is None:
        _CACHED = _build_kernel()
    nc = _CACHED

    bf = ml_dtypes.bfloat16
    cosT, sinT = _rope_tables()
    bmask = _bmask()
    x = np.asarray(x)
    W_q, W_k, W_v, W_o = (np.asarray(w) for w in (W_q, W_k, W_v, W_o))
    xT = [np.ascontiguousarray(x[b].T).astype(bf) for b in range(B)]

    in_maps = []
    for c in range(N_CORES):
        b, g = divmod(c, 4)
        cols = slice(DQ * g, DQ * (g + 1))
        in_maps.append(
            {
                "xT": xT[b],
                "wq": _swizzle(W_q[:, cols].astype(bf)),
                "wk": _swizzle(W_k[:, cols].astype(bf)),
                "wv": _swizzle(W_v[:, cols].astype(bf)),
                "wo": _swizzle(W_o[cols, :].astype(bf)),
                "cosT": cosT,
                "sinT": sinT,
                "bmask": bmask,
            }
        )

    res = run_bass_kernel_spmd(nc, in_maps, core_ids=list(range(N_CORES)))
    kernel.last_results = res

    y = np.empty((B, S, D), dtype=np.float32)
    for b in range(B):
        acc = res.results[4 * b]["yT"].astype(np.float32)
        for g in range(1, 4):
            acc += res.results[4 * b + g]["yT"].astype(np.float32)
        y[b] = acc.T
    return y
